# revision 1
# baseline (speedup 1.0000x reference)
"""Multi-head graph-attention (GAT) kernel for Trainium2, 8 NeuronCores.

Reference computation (per head):
    h_prime = h @ w[head]                       # [8192, 64]
    s = h_prime @ a_src[head],  d = h_prime @ a_dst[head]
    attn = softmax_j(leaky_relu(s_i + d_j, 0.2))
    out  = attn @ h_prime + bias                # -> [8192, 4*64]

Key identity: with exp monotone,
    exp(lrelu(s_i + d_j)) = e^{s_i} e^{d_j}           if s_i + d_j >= 0
                          = e^{0.2 s_i} e^{0.2 d_j}   otherwise
so with the 0/1 mask M[j,i] = [s_i + d_j >= 0] and row-scaled matrices
Hv = h' * e^d, Hq = h' * e^{0.2 d} (and vectors v = e^d, q = e^{0.2 d}):
    num[:,i]  = e^{s_i} (Hv^T M)[:,i] + e^{0.2 s_i} (Sq - (Hq^T M)[:,i])
    den[i]    = e^{s_i} (v^T M)[i]   + e^{0.2 s_i} (sum(q) - (q^T M)[i])
Only the mask is an O(n^2) elementwise op (one DVE tensor_scalar per tile);
the O(n^2) contraction runs on the PE as fp16 matmuls over the mask
(fp16's 10-bit mantissa keeps the result ~3e-5 relative; the fp32
corrections Sq / sum(q) are applied exactly in the epilogue):
  - mm1: stationary [Hv | Hq] fp16 ([128j, 128]) x mask -> AC psum
  - mm2: stationary [v | -q]  fp16 ([128j, 2])   x mask -> den psum
Two i-chunks are processed per stationary load: the second matmul of each
pair sets InstMatmult.ldweights=False to reuse the already-loaded weights
(LDWEIGHTS is serial with MATMUL on TRN2, ~95ns per 128-col fp16 load).

Sharding: 8 cores = 4 heads x 2 row-halves (head parallel + bs row shard).
Each core gets full h (2 MB) plus its row block; no collectives.
"""

import numpy as np

import concourse.bass as bass
import concourse.tile as tile
from concourse import bacc, mybir
from concourse.bass_utils import run_bass_kernel_spmd
from concourse.masks import make_identity

F32 = mybir.dt.float32
BF16 = mybir.dt.bfloat16
FP16 = mybir.dt.float16
AF = mybir.ActivationFunctionType
OP = mybir.AluOpType

BS = 8192          # nodes
F = 64             # f_in == f_out
NH = 4             # heads
HALF = BS // 2     # rows per core (row-half)
NT_J = BS // 128   # 64 j tiles
NT_I = HALF // 128 # 32 i tiles
NCH = HALF // 512  # 8 i chunks of 512
ALPHA = 0.2


def _build_kernel_module():
    nc = bacc.Bacc("TRN2", target_bir_lowering=False, debug=False)

    hfull_d = nc.dram_tensor("hfull", [BS, F], F32, kind="ExternalInput")
    hblk_d = nc.dram_tensor("hblk", [HALF, F], F32, kind="ExternalInput")
    w_d = nc.dram_tensor("w", [F, F], F32, kind="ExternalInput")
    aa_d = nc.dram_tensor("aa", [F, 2], F32, kind="ExternalInput")
    bias_d = nc.dram_tensor("bias", [1, F], F32, kind="ExternalInput")
    out_d = nc.dram_tensor("out", [HALF, F], F32, kind="ExternalOutput")

    with tile.TileContext(nc) as tc:
        with (
            tc.tile_pool(name="const", bufs=1) as cpool,
            tc.tile_pool(name="work", bufs=3) as wpool,
            tc.tile_pool(name="psum", bufs=2, space="PSUM") as ppool,
        ):
            # ---------------- constants ----------------
            identity = cpool.tile([128, 128], F32)
            make_identity(nc, identity[:])
            ones = cpool.tile([128, 512], F32)
            nc.gpsimd.memset(ones[:], 1.0)

            # ---------------- tiny weight prep ----------------
            w_sb = cpool.tile([F, F], F32)
            nc.sync.dma_start(w_sb[:], w_d.ap())
            aa_sb = cpool.tile([F, 2], F32)
            nc.sync.dma_start(aa_sb[:], aa_d.ap())
            bias_sb = cpool.tile([1, F], F32)
            nc.sync.dma_start(bias_sb[:], bias_d.ap())

            wT_ps = ppool.tile([F, F], F32, tag="mix")
            nc.tensor.transpose(wT_ps[:], w_sb[:], identity[0:F, 0:F])
            wT_sb = cpool.tile([F, F], F32)
            nc.scalar.copy(wT_sb[:], wT_ps[:])

            # ws = w @ [a_src | a_dst]  -> [64, 2]
            ws_ps = ppool.tile([F, 2], F32, tag="mix")
            nc.tensor.matmul(ws_ps[:], wT_sb[:], aa_sb[:])
            ws_sb = cpool.tile([F, 2], F32)
            nc.scalar.copy(ws_sb[:], ws_ps[:])

            # w_aug = [w | w@a_dst]  (h @ w_aug gives h_prime and d at once)
            w_aug = cpool.tile([F, F + 1], F32)
            nc.scalar.copy(w_aug[:, 0:F], w_sb[:])
            nc.scalar.copy(w_aug[:, F : F + 1], ws_sb[:, 1:2])

            # bias broadcast to all partitions
            biasb_ps = ppool.tile([128, F], F32, tag="mix")
            nc.tensor.matmul(biasb_ps[:], ones[0:1, 0:128], bias_sb[:])
            bias_rep = cpool.tile([128, F], F32)
            nc.scalar.copy(bias_rep[:], biasb_ps[:])

            # ---------------- row-block path first (feeds masks early) -------
            hbT = cpool.tile([F, HALF], F32)   # (row block of h)^T
            hb_view = hblk_d.ap().rearrange("(a p) f -> p a f", p=128)
            for blk in range(NT_I // 8):
                ldb = wpool.tile([128, 8 * F], F32, tag="hloadb", bufs=2)
                nc.sync.dma_start(
                    ldb[:], hb_view[:, blk * 8 : (blk + 1) * 8, :]
                )
                for k in range(8):
                    it = blk * 8 + k
                    tr = ppool.tile([F, 128], F32, tag="mix")
                    nc.tensor.transpose(
                        tr[:], ldb[:, k * F : (k + 1) * F], identity[:]
                    )
                    if k % 2 == 0:
                        nc.scalar.copy(hbT[:, it * 128 : (it + 1) * 128], tr[:])
                    else:
                        nc.vector.tensor_copy(
                            hbT[:, it * 128 : (it + 1) * 128], tr[:]
                        )

            s_row = cpool.tile([1, HALF], F32)
            for ch in range(NCH):
                sr_ps = ppool.tile([1, 512], F32, tag="mix")
                nc.tensor.matmul(
                    sr_ps[:], ws_sb[:, 0:1], hbT[:, ch * 512 : (ch + 1) * 512]
                )
                nc.scalar.copy(s_row[:, ch * 512 : (ch + 1) * 512], sr_ps[:])

            s_col = cpool.tile([128, NT_I], F32)
            for it in range(NT_I):
                sc_ps = ppool.tile([128, 1], F32, tag="mix")
                nc.tensor.matmul(
                    sc_ps[:], hbT[:, it * 128 : (it + 1) * 128], ws_sb[:, 0:1]
                )
                nc.scalar.copy(s_col[:, it : it + 1], sc_ps[:])

            u_col = cpool.tile([128, NT_I], F32)   # e^s
            nc.scalar.activation(u_col[:], s_col[:], AF.Exp)
            p_col = cpool.tile([128, NT_I], F32)   # e^{0.2 s}
            nc.scalar.activation(p_col[:], s_col[:], AF.Exp, scale=ALPHA)
            np_col = cpool.tile([128, NT_I], F32)  # -e^{0.2 s}
            nc.vector.tensor_scalar_mul(np_col[:], p_col[:], -1.0)

            # s replicated across partitions, bf16 (mask input)
            s_rep_b = cpool.tile([128, HALF], BF16)
            for ch in range(NCH):
                sb_ps = ppool.tile([128, 512], F32, tag="mix")
                nc.tensor.matmul(
                    sb_ps[:],
                    ones[0:1, 0:128],
                    s_row[:, ch * 512 : (ch + 1) * 512],
                )
                nc.scalar.copy(s_rep_b[:, ch * 512 : (ch + 1) * 512], sb_ps[:])

            # ---------------- h^T and h_prime (+d) for all j ----------------
            hT = cpool.tile([F, BS], F32)      # h^T, K-major for PE
            hpr = cpool.tile([128, NT_J * F], F32)     # h_prime, fp32
            hpr3 = hpr[:].rearrange("p (t c) -> p t c", c=F)
            d_col = cpool.tile([128, NT_J], F32)
            hf_view = hfull_d.ap().rearrange("(a p) f -> p a f", p=128)
            for blk in range(NT_J // 8):
                ldb = wpool.tile([128, 8 * F], F32, tag="hloadb", bufs=2)
                nc.sync.dma_start(
                    ldb[:], hf_view[:, blk * 8 : (blk + 1) * 8, :]
                )
                for k in range(8):
                    jt = blk * 8 + k
                    tr = ppool.tile([F, 128], F32, tag="mix")
                    nc.tensor.transpose(
                        tr[:], ldb[:, k * F : (k + 1) * F], identity[:]
                    )
                    if k % 2 == 0:
                        nc.scalar.copy(hT[:, jt * 128 : (jt + 1) * 128], tr[:])
                    else:
                        nc.vector.tensor_copy(
                            hT[:, jt * 128 : (jt + 1) * 128], tr[:]
                        )
                    hp_ps = ppool.tile([128, F + 1], F32, tag="mix")
                    nc.tensor.matmul(
                        hp_ps[:], hT[:, jt * 128 : (jt + 1) * 128], w_aug[:]
                    )
                    if k % 2 == 0:
                        nc.vector.tensor_copy(hpr3[:, jt, :], hp_ps[:, 0:F])
                    else:
                        nc.scalar.copy(hpr3[:, jt, :], hp_ps[:, 0:F])
                    nc.vector.tensor_copy(
                        d_col[:, jt : jt + 1], hp_ps[:, F : F + 1]
                    )

            # per-16-tile groups: exps of d, VQ and HH builds (lets the main
            # loop start as soon as the first tiles are ready)
            GRP = 16
            v_col = cpool.tile([128, NT_J], F32)
            q_col = cpool.tile([128, NT_J], F32)
            nq_col = cpool.tile([128, NT_J], F32)
            negd_col = cpool.tile([128, NT_J], F32)
            VQ = cpool.tile([128, NT_J * 2], FP16)
            VQ3 = VQ[:].rearrange("p (t c) -> p t c", c=2)
            HH = cpool.tile([128, NT_J * 128], FP16)
            HH3 = HH[:].rearrange("p (t c) -> p t c", c=128)
            sq_ps = ppool.tile([1, F], F32, tag="sq", bufs=1)
            for g in range(NT_J // GRP):
                gs = slice(g * GRP, (g + 1) * GRP)
                nc.scalar.activation(v_col[:, gs], d_col[:, gs], AF.Exp)
                nc.scalar.activation(q_col[:, gs], d_col[:, gs], AF.Exp, scale=ALPHA)
                nc.vector.tensor_scalar_mul(nq_col[:, gs], q_col[:, gs], -1.0)
                nc.vector.tensor_scalar_mul(negd_col[:, gs], d_col[:, gs], -1.0)
                nc.vector.tensor_copy(VQ3[:, gs, 0], v_col[:, gs])
                nc.vector.tensor_copy(VQ3[:, gs, 1], nq_col[:, gs])
                for jt in range(g * GRP, (g + 1) * GRP):
                    stage = wpool.tile([128, 128], F32, tag="stage", bufs=3)
                    nc.vector.tensor_scalar_mul(
                        stage[:, 0:F], hpr3[:, jt, :], v_col[:, jt : jt + 1]
                    )
                    nc.vector.tensor_scalar_mul(
                        stage[:, F:128], hpr3[:, jt, :], q_col[:, jt : jt + 1]
                    )
                    nc.scalar.copy(HH3[:, jt, :], stage[:])
                    # Sq accumulation: ones^T @ Hq_f32
                    nc.tensor.matmul(
                        sq_ps[:],
                        ones[:, 0:1],
                        stage[:, F:128],
                        start=(jt == 0),
                        stop=(jt == NT_J - 1),
                    )
            sq_sb = cpool.tile([1, F], F32)
            nc.scalar.copy(sq_sb[:], sq_ps[:])
            # Sq broadcast to all partitions (fp32 exact correction)
            sqb_ps = ppool.tile([128, F], F32, tag="mix")
            nc.tensor.matmul(sqb_ps[:], ones[0:1, 0:128], sq_sb[:])
            Sq_rep = cpool.tile([128, F], F32)
            nc.scalar.copy(Sq_rep[:], sqb_ps[:])

            # Sq_tot = sum_j q_j, then pSqt[:, it] = p * Sq_tot (epilogue bias)
            qs_ps = ppool.tile([NT_J, 1], F32, tag="sq", bufs=1)
            nc.tensor.matmul(qs_ps[:], q_col[:], ones[:, 0:1])
            qs_sb = cpool.tile([NT_J, 1], F32)
            nc.scalar.copy(qs_sb[:], qs_ps[:])
            sqt_ps = ppool.tile([1, 1], F32, tag="sq", bufs=1)
            nc.tensor.matmul(sqt_ps[:], qs_sb[:], ones[0:NT_J, 0:1])
            sqt_sb = cpool.tile([1, 1], F32)
            nc.scalar.copy(sqt_sb[:], sqt_ps[:])
            sqtb_ps = ppool.tile([128, 1], F32, tag="sq", bufs=1)
            nc.tensor.matmul(sqtb_ps[:], ones[0:1, 0:128], sqt_sb[:])
            Sqt_col = cpool.tile([128, 1], F32)
            nc.scalar.copy(Sqt_col[:], sqtb_ps[:])
            pSqt = cpool.tile([128, NT_I], F32)
            nc.vector.tensor_scalar_mul(pSqt[:], p_col[:], Sqt_col[:])

            # ---------------- main flash loop (software-pipelined) -----------
            def epilogue(ch, AC_sb, den_sb, te=None):
                te = te or nc.gpsimd
                for sub in range(4):
                    it = ch * 4 + sub
                    ACt_ps = ppool.tile([128, 128], F32, tag="mix")
                    nc.tensor.transpose(
                        ACt_ps[:],
                        AC_sb[:, sub * 128 : (sub + 1) * 128],
                        identity[:],
                    )
                    dent_ps = ppool.tile([128, 2], F32, tag="mix")
                    nc.tensor.transpose(
                        dent_ps[:],
                        den_sb[:, sub * 128 : (sub + 1) * 128],
                        identity[0:2, 0:2],
                    )
                    # numerator = u*A_T + p*Sq - p*C_T
                    t1 = wpool.tile([128, F], F32, tag="t1", bufs=2)
                    nc.scalar.activation(
                        t1[:], ACt_ps[:, 0:F], AF.Identity,
                        scale=u_col[:, it : it + 1],
                    )
                    cT = wpool.tile([128, F], F32, tag="cT", bufs=2)
                    nc.scalar.activation(
                        cT[:], ACt_ps[:, F:128], AF.Identity,
                        scale=np_col[:, it : it + 1],
                    )
                    pSq = wpool.tile([128, F], F32, tag="pSq", bufs=2)
                    te.tensor_scalar_mul(
                        pSq[:], Sq_rep[:], p_col[:, it : it + 1]
                    )
                    n1 = wpool.tile([128, F], F32, tag="n1", bufs=2)
                    te.tensor_add(n1[:], t1[:], cT[:])
                    num = wpool.tile([128, F], F32, tag="num", bufs=2)
                    te.tensor_add(num[:], n1[:], pSq[:])
                    # denominator = u*(vM) + p*Sqt - p*(qM)
                    y1 = wpool.tile([128, 1], F32, tag="y1", bufs=2)
                    nc.scalar.activation(
                        y1[:], dent_ps[:, 0:1], AF.Identity,
                        scale=u_col[:, it : it + 1],
                    )
                    y2 = wpool.tile([128, 1], F32, tag="y2", bufs=2)
                    nc.scalar.activation(
                        y2[:], dent_ps[:, 1:2], AF.Identity,
                        scale=p_col[:, it : it + 1],
                        bias=pSqt[:, it : it + 1],
                    )
                    den = wpool.tile([128, 1], F32, tag="den", bufs=2)
                    te.tensor_add(den[:], y1[:], y2[:])
                    rec = wpool.tile([128, 1], F32, tag="rec", bufs=2)
                    nc.vector.reciprocal(rec[:], den[:])
                    o_t = wpool.tile([128, F], F32, tag="ot", bufs=2)
                    nc.scalar.activation(
                        o_t[:], num[:], AF.Identity, scale=rec[:]
                    )
                    o_f = wpool.tile([128, F], F32, tag="of", bufs=2)
                    te.tensor_add(o_f[:], o_t[:], bias_rep[:])
                    nc.sync.dma_start(
                        out_d.ap()[it * 128 : (it + 1) * 128, :], o_f[:]
                    )

            pending = []
            for grp in range(NCH // 4):
                while pending:
                    epilogue(*pending.pop(0))
                ACs = [
                    ppool.tile([128, 512], F32, tag="acc", bufs=4, name=f"AC{c}")
                    for c in range(4)
                ]
                den4 = ppool.tile([98, 512], F32, tag="dacc", bufs=1)
                chs = [4 * grp + c for c in range(4)]
                for jt in range(NT_J):
                    ms = []
                    for half2 in range(2):
                        mw = wpool.tile([128, 1024], FP16, tag="mask", bufs=4,
                                        name=f"mw{half2}")
                        c0 = chs[2 * half2]
                        nc.vector.tensor_scalar(
                            mw[:], s_rep_b[:, c0 * 512 : (c0 + 2) * 512],
                            negd_col[:, jt : jt + 1], None, OP.is_ge,
                        )
                        ms.append(mw[:, 0:512])
                        ms.append(mw[:, 512:1024])
                    st, sp = (jt == 0), (jt == NT_J - 1)
                    for c in range(4):
                        i = nc.tensor.matmul(
                            ACs[c][:], HH3[:, jt, :], ms[c][:], start=st, stop=sp
                        )
                        if c > 0:
                            i.ins.ldweights = False
                    for c in range(4):
                        nc.tensor.matmul(
                            den4[32 * c : 32 * c + 2, :], VQ3[:, jt, :], ms[c][:],
                            start=st, stop=sp, tile_position=(0, 32 * c),
                        )
                for c in range(4):
                    AC_sb = wpool.tile([128, 512], F32, tag="ACsb", bufs=5,
                                       name=f"ACsb{c}")
                    nc.scalar.copy(AC_sb[:], ACs[c][:])
                    den_sb = wpool.tile([2, 512], F32, tag="densb", bufs=5,
                                        name=f"densb{c}")
                    nc.scalar.copy(den_sb[:], den4[32 * c : 32 * c + 2, :])
                    pending.append((chs[c], AC_sb, den_sb[:]))
            for k, args in enumerate(pending):
                epilogue(*args, te=(nc.vector if k % 2 == 0 else nc.gpsimd))

    nc.compile()
    return nc


_NC_CACHE = None


def _get_nc():
    global _NC_CACHE
    if _NC_CACHE is None:
        _NC_CACHE = _build_kernel_module()
    return _NC_CACHE


def _make_in_maps(h, w, a_src, a_dst, bias):
    h = np.ascontiguousarray(np.asarray(h, dtype=np.float32))
    w = np.asarray(w, dtype=np.float32)
    a_src = np.asarray(a_src, dtype=np.float32)
    a_dst = np.asarray(a_dst, dtype=np.float32)
    bias = np.asarray(bias, dtype=np.float32).reshape(1, F)
    in_maps = []
    for c in range(8):
        head, half = c // 2, c % 2
        aa = np.ascontiguousarray(
            np.concatenate([a_src[head], a_dst[head]], axis=1)
        )
        in_maps.append(
            {
                "hfull": h,
                "hblk": np.ascontiguousarray(h[half * HALF : (half + 1) * HALF]),
                "w": np.ascontiguousarray(w[head]),
                "aa": aa,
                "bias": bias,
            }
        )
    return in_maps


def _run(h, w, a_src, a_dst, bias, trace=False, **trace_kwargs):
    nc = _get_nc()
    in_maps = _make_in_maps(h, w, a_src, a_dst, bias)
    res = run_bass_kernel_spmd(
        nc, in_maps, core_ids=list(range(8)), trace=trace, **trace_kwargs
    )
    out = np.zeros((BS, NH * F), dtype=np.float32)
    for c in range(8):
        head, half = c // 2, c % 2
        out[half * HALF : (half + 1) * HALF, head * F : (head + 1) * F] = res.results[
            c
        ]["out"]
    return out, res


def kernel(h, w, a_src, a_dst, bias):
    out, _ = _run(h, w, a_src, a_dst, bias, trace=False)
    return out



# revision 10
# speedup vs baseline: 3.8346x; 3.8346x over previous
"""Multi-head graph-attention (GAT) kernel for Trainium2, 8 NeuronCores.

Reference computation (per head):
    h_prime = h @ w[head]                       # [8192, 64]
    s = h_prime @ a_src[head],  d = h_prime @ a_dst[head]
    attn = softmax_j(leaky_relu(s_i + d_j, 0.2))
    out  = attn @ h_prime + bias                # -> [8192, 4*64]

Low-rank reformulation (no O(n^2) work on device):
    W[i,j] = exp(lrelu(s_i + d_j)) = e^{s_i} e^{d_j} + K(s_i, d_j)
  where K(s,d) = exp(0.2(s+d)) - exp(s+d) for s+d < 0, else 0 is a bounded
  continuous function on the (s,d) rectangle covered by the data.  K is
  approximated by a rank-R Chebyshev product expansion (R=32):
    K(s,d) ~= sum_{a,b} c_ab T_a(s/Ls) T_b(d/Ld)
  fitted ON THE HOST per head (coefficients + ranges are runtime inputs).
  Then with Haug = [h' | s^ | d^ | 1]:
    B  = TDaug^T @ Haug    TDaug = [T_0..T_31 | e^d]      (PE, O(n R))
    BK = C^T @ B[T-rows];  BKaug = [BK-rows | e^d-row]
    nd = BKaug^T @ Faug    Faug  = [T_0..T_31 | e^s]^T    (PE, O(n R))
  nd rows 0..63 are the softmax numerator, row 66 the denominator; the
  epilogue transposes, divides, adds bias and stores.  Max rel err vs the
  fp64 reference is ~1e-3 (fp16 operands, fp32 psum accumulation), well
  inside the 2e-2 gate.

Sharding: 8 cores = 4 heads x 2 row-halves (head parallel + bs row shard).
Each core gets full h (2 MB) plus per-head host-fitted constants; no
collectives.  The j-side (all 8192 nodes) is processed once per core; the
i-side covers only the core's 4096-row half.
"""

import numpy as np

import concourse.bass as bass
import concourse.tile as tile
from concourse import bacc, mybir
from concourse.bass_utils import run_bass_kernel_spmd
from concourse.masks import make_identity

F32 = mybir.dt.float32
F16 = mybir.dt.float16
AF = mybir.ActivationFunctionType
OP = mybir.AluOpType

BS = 8192          # nodes
F = 64             # f_in == f_out
NH = 4             # heads
HALF = BS // 2     # rows per core (row-half)
NT = BS // 128     # 64 j tiles
NTI = HALF // 128  # 32 i tiles
R = 32             # Chebyshev rank
MB = R + 1         # TDaug / Faug columns: T_0..T_{R-1}, e^d (resp e^s)
MH = 67            # Haug columns: h'(64), s^, d^, ones
ALPHA = 0.2
ND_SCALE = 2.0 ** -8   # scale on BKaug so num/den fit fp16 in the epilogue


def _build_kernel_module():
    nc = bacc.Bacc("TRN2", target_bir_lowering=False, debug=False)

    h_d = nc.dram_tensor("hfull", [BS, F], F32, kind="ExternalInput")
    # waug: [w | (w@a_src)/Ls | (w@a_dst)/Ld]
    waug_d = nc.dram_tensor("waug", [F, F + 2], F32, kind="ExternalInput")
    # ct: C^T [R, R] (Chebyshev coefficients of K, transposed)
    ct_d = nc.dram_tensor("ct", [R, R], F32, kind="ExternalInput")
    # scal: [128, 2] (Ls, Ld) replicated across partitions
    scal_d = nc.dram_tensor("scal", [128, 2], F32, kind="ExternalInput")
    # bias replicated across partitions
    biasr_d = nc.dram_tensor("biasr", [128, F], F32, kind="ExternalInput")
    # which half this core owns (0 or 1), replicated; only jt offset differs
    out_d = nc.dram_tensor("out", [HALF, F], F32, kind="ExternalOutput")
    # jt offset of the i-half is passed via a separate input so that the
    # compiled module stays identical across cores: instead we just compile
    # the half-offset in two variants? -> simpler: the host rotates h so
    # every core's own half sits in j-tiles 0..31 (see _make_in_maps).

    with tile.TileContext(nc) as tc:
        with (
            tc.tile_pool(name="const", bufs=1) as cpool,
            tc.tile_pool(name="work", bufs=3) as wpool,
            tc.tile_pool(name="psum", bufs=2, space="PSUM") as ppool,
        ):
            # ---------------- constants ----------------
            ident16 = cpool.tile([128, 128], F16)
            make_identity(nc, ident16[:])

            waug_sb = cpool.tile([F, F + 2], F32)
            nc.sync.dma_start(waug_sb[:], waug_d.ap())
            waug16 = cpool.tile([F, F + 2], F16)
            nc.gpsimd.tensor_copy(waug16[:], waug_sb[:])

            ct_sb = cpool.tile([R, R], F32)
            nc.sync.dma_start(ct_sb[:], ct_d.ap())
            ct16 = cpool.tile([R, R], F16)
            nc.gpsimd.tensor_copy(ct16[:], ct_sb[:])

            scal_sb = cpool.tile([128, 2], F32)
            nc.sync.dma_start(scal_sb[:], scal_d.ap())
            biasr = cpool.tile([128, F], F32)
            nc.sync.dma_start(biasr[:], biasr_d.ap())

            # ---------------- big SBUF tensors ----------------
            hT = cpool.tile([F, BS], F16)               # h^T (K-major), fp16
            Haug = cpool.tile([128, NT * MH], F16)      # [h' | s^ | d^ | 1]
            Haug3 = Haug[:].rearrange("p (t c) -> p t c", c=MH)
            sd = cpool.tile([128, NT * 2], F32)         # [s^, d^] per j tile
            sd3 = sd[:].rearrange("p (t c) -> p t c", c=2)
            TDaug = cpool.tile([128, NT * MB], F16)     # [T_0..T_31 | e^d]
            TDaug3 = TDaug[:].rearrange("p (t c) -> p t c", c=MB)
            T_all = cpool.tile([128, R * NT], F32)      # fp32 recurrence state
            T3 = T_all[:].rearrange("p (b t) -> p b t", t=NT)
            Fi = cpool.tile([128, NTI * MB], F16)       # i-side [T_a | e^s]
            Fi3 = Fi[:].rearrange("p (t c) -> p t c", c=MB)
            S_all = cpool.tile([128, R * NTI], F32)
            S3 = S_all[:].rearrange("p (b t) -> p b t", t=NTI)
            F3t = cpool.tile([MB, NTI * 128], F16)      # Faug^T for synthesis
            F33 = F3t[:].rearrange("p (t c) -> p t c", c=128)

            # ones columns (T_0 = 1 and Haug ones col); zero the s/d slots of
            # Haug so B's columns 64/65 are exactly 0 (never overflow fp16)
            nc.gpsimd.memset(Haug3[:, :, F : F + 2], 0.0)
            nc.gpsimd.memset(Haug3[:, :, MH - 1], 1.0)
            nc.gpsimd.memset(TDaug3[:, :, 0], 1.0)
            nc.gpsimd.memset(Fi3[:, :, 0], 1.0)

            def cp(e, out, in_):
                (e.copy if e is nc.scalar else e.tensor_copy)(out, in_)

            # ---------------- phase 1: load h, transpose, h' ----------------
            # PSUM-reading copies may only run on vector/scalar (GPSIMD has no
            # PSUM access); they are batched 4 j-tiles at a time.
            hview = h_d.ap().rearrange("(a p) f -> p a f", p=128)
            copy_engines = [nc.vector, nc.scalar, nc.gpsimd]
            ps_engines = [nc.vector, nc.scalar]
            for blk in range(NT // 8):
                ldb = wpool.tile([128, 8 * F], F32, tag="ldb", bufs=2)
                nc.sync.dma_start(ldb[:], hview[:, blk * 8 : (blk + 1) * 8, :])
                ldb16 = wpool.tile([128, 8 * F], F16, tag="ldb16", bufs=2)
                cp(copy_engines[2 - blk % 2], ldb16[:], ldb[:])
                for g in range(2):
                    jt0 = blk * 8 + g * 4
                    trp4 = ppool.tile([F, 4 * 128], F16, tag="mix", bufs=4)
                    for k in range(4):
                        nc.tensor.transpose(
                            trp4[:, k * 128 : (k + 1) * 128],
                            ldb16[:, (g * 4 + k) * F : (g * 4 + k + 1) * F],
                            ident16[:],
                        )
                    cp(ps_engines[jt0 // 4 % 2],
                       hT[:, jt0 * 128 : (jt0 + 4) * 128], trp4[:])
                    hp4 = ppool.tile([128, 4 * (F + 2)], F32, tag="mix", bufs=4)
                    hp43 = hp4[:].rearrange("p (t c) -> p t c", c=F + 2)
                    for k in range(4):
                        jt = jt0 + k
                        nc.tensor.matmul(
                            hp43[:, k, :],
                            hT[:, jt * 128 : (jt + 1) * 128],
                            waug16[:],
                        )
                    cp(ps_engines[(jt0 // 4 + 1) % 2],
                       Haug3[:, jt0 : jt0 + 4, 0:F], hp43[:, :, 0:F])
                    cp(ps_engines[jt0 // 4 % 2],
                       sd3[:, jt0 : jt0 + 4, :], hp43[:, :, F : F + 2])

            # ---------------- phase 2: Chebyshev on d (j side) ----------------
            d_view = sd3[:, :, 1]                # [128, NT] strided f32
            s_view = sd3[:, 0:NTI, 0]            # [128, NTI] own half

            # e^d  (scale by Ld: d^ * Ld = d)
            nc.scalar.activation(
                TDaug3[:, :, MB - 1], d_view, AF.Exp, scale=scal_sb[:, 1:2]
            )
            # T_1 = d^
            nc.vector.tensor_copy(TDaug3[:, :, 1], d_view)
            d2 = cpool.tile([128, NT], F32)
            nc.vector.tensor_scalar_mul(d2[:], d_view, 2.0)
            # fp32 recurrence state: T3[:,0,:]=1, T3[:,1,:]=d^
            nc.gpsimd.memset(T3[:, 0, :], 1.0)
            nc.gpsimd.tensor_copy(T3[:, 1, :], d_view)
            for b in range(2, R):
                tmp = wpool.tile([128, NT], F32, tag="tmp", bufs=2)
                nc.vector.tensor_mul(tmp[:], d2[:], T3[:, b - 1, :])
                nc.vector.tensor_sub(T3[:, b, :], tmp[:], T3[:, b - 2, :])
                cp(copy_engines[b % 2 + 1], TDaug3[:, :, b], T3[:, b, :])

            # ---------------- phase 2b: Chebyshev on s (i side) ----------------
            nc.scalar.activation(
                Fi3[:, :, MB - 1], s_view, AF.Exp, scale=scal_sb[:, 0:1]
            )
            nc.vector.tensor_copy(Fi3[:, :, 1], s_view)
            s2 = cpool.tile([128, NTI], F32)
            nc.vector.tensor_scalar_mul(s2[:], s_view, 2.0)
            nc.gpsimd.memset(S3[:, 0, :], 1.0)
            nc.gpsimd.tensor_copy(S3[:, 1, :], s_view)
            for b in range(2, R):
                tmp = wpool.tile([128, NTI], F32, tag="tmps", bufs=2)
                nc.vector.tensor_mul(tmp[:], s2[:], S3[:, b - 1, :])
                nc.vector.tensor_sub(S3[:, b, :], tmp[:], S3[:, b - 2, :])
                cp(copy_engines[b % 2 + 1], Fi3[:, :, b], S3[:, b, :])

            # ---------------- phase 3: B = TDaug^T @ Haug ----------------
            B_ps = ppool.tile([MB, MH], F32, tag="acc", bufs=1)
            for jt in range(NT):
                nc.tensor.matmul(
                    B_ps[:], TDaug3[:, jt, :], Haug3[:, jt, :],
                    start=(jt == 0), stop=(jt == NT - 1),
                )
            B16 = cpool.tile([MB, MH], F16)
            nc.scalar.copy(B16[:], B_ps[:])

            # BK = C^T-weighted combination; BKaug = [BK rows | e^d row]
            bk_ps = ppool.tile([R, MH], F32, tag="mix", bufs=4)
            nc.tensor.matmul(bk_ps[:], ct16[:], B16[0:R, :])
            BKaug = cpool.tile([MB, MH], F16)
            nc.vector.tensor_scalar_mul(BKaug[0:R, :], bk_ps[:], ND_SCALE)
            nc.vector.tensor_scalar_mul(
                BKaug[R : R + 1, :], B16[R : R + 1, :], ND_SCALE
            )

            # ---------------- phase 4: transpose Faug ----------------
            for it0 in range(0, NTI, 2):
                ftp2 = ppool.tile([MB, 256], F16, tag="mix", bufs=4)
                for k in range(2):
                    nc.tensor.transpose(
                        ftp2[:, k * 128 : (k + 1) * 128],
                        Fi3[:, it0 + k, :], ident16[:],
                    )
                cp(ps_engines[it0 // 2 % 2],
                   F33[:, it0 : it0 + 2, :], ftp2[:])

            # ---------------- phase 5: synthesis + epilogue ----------------
            for ch in range(NTI // 4):
                nd_ps = ppool.tile([MH, 512], F32, tag="nd", bufs=2)
                nc.tensor.matmul(
                    nd_ps[:], BKaug[:], F3t[:, ch * 512 : (ch + 1) * 512]
                )
                nd_sb = wpool.tile([MH, 512], F16, tag="ndsb", bufs=2)
                cp(copy_engines[ch % 2], nd_sb[:], nd_ps[:])
                for sub in range(4):
                    it = ch * 4 + sub
                    ot_ps = ppool.tile([128, MH], F16, tag="mix", bufs=4)
                    nc.tensor.transpose(
                        ot_ps[:],
                        nd_sb[:, sub * 128 : (sub + 1) * 128],
                        ident16[0:MH, 0:MH],
                    )
                    rec = wpool.tile([128, 1], F32, tag="rec", bufs=3)
                    nc.vector.reciprocal(rec[:], ot_ps[:, MH - 1 : MH])
                    o1 = wpool.tile([128, F], F32, tag="o1", bufs=3)
                    if sub % 2 == 0:
                        nc.scalar.mul(o1[:], ot_ps[:, 0:F], rec[:])
                    else:
                        nc.vector.tensor_scalar_mul(o1[:], ot_ps[:, 0:F], rec[:])
                    o2 = wpool.tile([128, F], F32, tag="o2", bufs=3)
                    (nc.gpsimd if sub % 2 == 0 else nc.vector).tensor_add(
                        o2[:], o1[:], biasr[:]
                    )
                    nc.sync.dma_start(
                        out_d.ap()[it * 128 : (it + 1) * 128, :], o2[:]
                    )

    nc.compile()
    return nc


_NC_CACHE = None


def _get_nc():
    global _NC_CACHE
    if _NC_CACHE is None:
        _NC_CACHE = _build_kernel_module()
    return _NC_CACHE


def _cheb_fit_K(Ls, Ld, Ra, Rb, ngrid=96):
    """Chebyshev product coefficients of K(s,d) = exp(.2(s+d))-exp(s+d) (s+d<0)
    on [-Ls,Ls] x [-Ld,Ld].  Returns C [Ra, Rb]."""
    ks = np.cos((np.arange(ngrid) + 0.5) * np.pi / ngrid)
    S, D = np.meshgrid(ks * Ls, ks * Ld, indexing="ij")
    X = S + D
    K = np.where(X < 0, np.exp(ALPHA * X) - np.exp(X), 0.0)
    Ts = np.polynomial.chebyshev.chebvander(ks, Ra - 1)   # [ngrid, Ra]
    Td = np.polynomial.chebyshev.chebvander(ks, Rb - 1)
    C = (2.0 / ngrid) ** 2 * np.einsum("ij,ia,jb->ab", K, Ts, Td)
    C[0, :] *= 0.5
    C[:, 0] *= 0.5
    return C


def _make_in_maps(h, w, a_src, a_dst, bias):
    h = np.ascontiguousarray(np.asarray(h, dtype=np.float32))
    w = np.asarray(w, dtype=np.float32)
    a_src = np.asarray(a_src, dtype=np.float32)
    a_dst = np.asarray(a_dst, dtype=np.float32)
    bias = np.asarray(bias, dtype=np.float32)
    biasr = np.ascontiguousarray(np.broadcast_to(bias[None, :], (128, F)))

    in_maps = []
    for c in range(8):
        head, half = c // 2, c % 2
        ws = w[head] @ a_src[head][:, 0]          # [64]
        wd = w[head] @ a_dst[head][:, 0]
        s_all = h @ ws
        d_all = h @ wd
        Ls = float(np.abs(s_all).max()) * 1.02 + 1e-30
        Ld = float(np.abs(d_all).max()) * 1.02 + 1e-30
        C = _cheb_fit_K(Ls, Ld, R, R)
        waug = np.concatenate(
            [w[head], (ws / Ls)[:, None], (wd / Ld)[:, None]], axis=1
        ).astype(np.float32)
        scal = np.broadcast_to(
            np.array([Ls, Ld], dtype=np.float32)[None, :], (128, 2)
        )
        # rotate h so this core's own half sits in j-tiles 0..31
        if half == 0:
            h_rot = h
        else:
            h_rot = np.concatenate([h[HALF:], h[:HALF]], axis=0)
        in_maps.append(
            {
                "hfull": np.ascontiguousarray(h_rot),
                "waug": np.ascontiguousarray(waug),
                "ct": np.ascontiguousarray(C.T.astype(np.float32)),
                "scal": np.ascontiguousarray(scal),
                "biasr": biasr,
            }
        )
    return in_maps


def _run(h, w, a_src, a_dst, bias, trace=False, **trace_kwargs):
    nc = _get_nc()
    in_maps = _make_in_maps(h, w, a_src, a_dst, bias)
    res = run_bass_kernel_spmd(
        nc, in_maps, core_ids=list(range(8)), trace=trace, **trace_kwargs
    )
    out = np.zeros((BS, NH * F), dtype=np.float32)
    for c in range(8):
        head, half = c // 2, c % 2
        out[half * HALF : (half + 1) * HALF, head * F : (head + 1) * F] = res.results[
            c
        ]["out"]
    return out, res


def kernel(h, w, a_src, a_dst, bias):
    out, _ = _run(h, w, a_src, a_dst, bias, trace=False)
    return out


# revision 12
# speedup vs baseline: 4.1816x; 1.0905x over previous
"""Multi-head graph-attention (GAT) kernel for Trainium2, 8 NeuronCores.

Reference computation (per head):
    h_prime = h @ w[head]                       # [8192, 64]
    s = h_prime @ a_src[head],  d = h_prime @ a_dst[head]
    attn = softmax_j(leaky_relu(s_i + d_j, 0.2))
    out  = attn @ h_prime + bias                # -> [8192, 4*64]

Low-rank reformulation (no O(n^2) work on device):
    W[i,j] = exp(lrelu(s_i + d_j)) = e^{s_i} e^{d_j} + K(s_i, d_j)
  where K(s,d) = exp(0.2(s+d)) - exp(s+d) for s+d < 0, else 0, is a bounded
  continuous function on the (s,d) rectangle covered by the data.  K is
  approximated by a rank-R Chebyshev product expansion (R=31):
    K(s,d) ~= sum_{a,b} c_ab T_a(s/Ls) T_b(d/Ld)
  fitted ON THE HOST per head (coefficients + ranges are runtime inputs).
  With Haug = [h' | 0 0 | 1] and TDaug = [T_0..T_30 | e^d] (j on partitions):
    B      = TDaug^T @ Haug                      # [32, 67]   (PE, O(n R))
    BKaug4 = E^T @ B                             # [128, 67]  one matmul; E is
             host-built so rows 32k+a give sum_b c_ab B_Tb + e^d row at 32k+31,
             replicated for k=0..3 and pre-scaled by 2^-8
    out^T  = Faug-tile^T @ BKaug4[32k:32k+32]    # [128 i, 67] per i-tile,
  where Faug = [T_0..T_30 | e^s] of the i side, transposed 4 tiles at a time
  (partition bands 0/32/64/96).  Row 66 of out^T is the softmax denominator;
  the epilogue divides, adds bias and stores.  Max rel err vs the reference is
  ~1e-3 (fp16 operands, fp32 psum accumulation), inside the 2e-2 gate.

Sharding: 8 cores = 4 heads x 2 row-halves (head parallel + bs row shard).
Each core gets full h (rotated so its own half is first) plus per-head
host-fitted constants; no collectives.
"""

import numpy as np

import concourse.bass as bass
import concourse.tile as tile
from concourse import bacc, mybir
from concourse.bass_utils import run_bass_kernel_spmd
from concourse.masks import make_identity

F32 = mybir.dt.float32
F16 = mybir.dt.float16
AF = mybir.ActivationFunctionType

BS = 8192          # nodes
F = 64             # f_in == f_out
NH = 4             # heads
HALF = BS // 2     # rows per core (row-half)
NT = BS // 128     # 64 j tiles
NTI = HALF // 128  # 32 i tiles
R = 31             # Chebyshev rank
MB = R + 1         # TDaug / Faug columns: T_0..T_{R-1}, e^d (resp e^s)
MH = 67            # Haug columns: h'(64), 2 zero, ones
ALPHA = 0.2
ND_SCALE = 2.0 ** -8   # folded into E so num/den stay small


def _build_kernel_module():
    nc = bacc.Bacc("TRN2", target_bir_lowering=False, debug=False)

    h_d = nc.dram_tensor("hfull", [BS, F], F32, kind="ExternalInput")
    # waug: [w | (w@a_src)/Ls | (w@a_dst)/Ld]
    waug_d = nc.dram_tensor("waug", [F, F + 2], F32, kind="ExternalInput")
    # E: [MB, 128] expansion matrix (Cheb coeffs -> 4x replicated BKaug rows)
    e_d = nc.dram_tensor("emat", [MB, 128], F32, kind="ExternalInput")
    # scal: [128, 2] (Ls, Ld) replicated across partitions
    scal_d = nc.dram_tensor("scal", [128, 2], F32, kind="ExternalInput")
    # bias replicated across partitions
    biasr_d = nc.dram_tensor("biasr", [128, F], F32, kind="ExternalInput")
    out_d = nc.dram_tensor("out", [HALF, F], F32, kind="ExternalOutput")

    with tile.TileContext(nc) as tc:
        with (
            tc.tile_pool(name="const", bufs=1) as cpool,
            tc.tile_pool(name="work", bufs=3) as wpool,
            tc.tile_pool(name="psum", bufs=2, space="PSUM") as ppool,
        ):
            # ---------------- constants ----------------
            ident32 = cpool.tile([128, 128], F32)
            make_identity(nc, ident32[:])
            ident16 = cpool.tile([128, 128], F16)
            nc.gpsimd.tensor_copy(ident16[:], ident32[:])

            # waug on partitions 0-63 AND 64-127 (for row-band hp matmuls)
            waug_sb = cpool.tile([128, F + 2], F32)
            nc.sync.dma_start(waug_sb[0:F, :], waug_d.ap())
            nc.sync.dma_start(waug_sb[F:128, :], waug_d.ap())
            waug16 = cpool.tile([128, F + 2], F16)
            nc.gpsimd.tensor_copy(waug16[:], waug_sb[:])

            e_sb = cpool.tile([MB, 128], F32)
            nc.sync.dma_start(e_sb[:], e_d.ap())
            e16 = cpool.tile([MB, 128], F16)
            nc.gpsimd.tensor_copy(e16[:], e_sb[:])

            scal_sb = cpool.tile([128, 2], F32)
            nc.sync.dma_start(scal_sb[:], scal_d.ap())
            biasr = cpool.tile([128, F], F32)
            nc.sync.dma_start(biasr[:], biasr_d.ap())

            # ---------------- big SBUF tensors ----------------
            # h^T stored as 2-tile blocks: block t holds j-tiles (2t, 2t+1) on
            # partition halves 0-63 / 64-127
            hT2 = cpool.tile([128, (NT // 2) * 128], F16)
            Haug = cpool.tile([128, NT * MH], F16)      # [h' | 0 0 | 1]
            Haug3 = Haug[:].rearrange("p (t c) -> p t c", c=MH)
            sd = cpool.tile([128, NT * 2], F32)         # [s^, d^] per j tile
            sd3 = sd[:].rearrange("p (t c) -> p t c", c=2)
            TDaug = cpool.tile([128, NT * MB], F16)     # [T_0..T_30 | e^d]
            TDaug3 = TDaug[:].rearrange("p (t c) -> p t c", c=MB)
            T_all = cpool.tile([128, R * NT], F32)      # fp32 recurrence state
            T3 = T_all[:].rearrange("p (b t) -> p b t", t=NT)
            Fi = cpool.tile([128, NTI * MB], F16)       # i-side [T_a | e^s]
            Fi3 = Fi[:].rearrange("p (t c) -> p t c", c=MB)
            S_all = cpool.tile([128, R * NTI], F32)
            S3 = S_all[:].rearrange("p (b t) -> p b t", t=NTI)
            # Faug^T: block q holds i-tiles 4q..4q+3 on partition bands 32k
            F3t = cpool.tile([128, (NTI // 4) * 128], F16)
            F33 = F3t[:].rearrange("p (t c) -> p t c", c=128)
            BKaug4 = cpool.tile([128, MH], F16)

            nc.gpsimd.memset(Haug3[:, :, F : F + 2], 0.0)
            nc.gpsimd.memset(Haug3[:, :, MH - 1], 1.0)
            nc.gpsimd.memset(TDaug3[:, :, 0], 1.0)
            nc.gpsimd.memset(Fi3[:, :, 0], 1.0)

            def cp(e, out, in_):
                (e.copy if e is nc.scalar else e.tensor_copy)(out, in_)

            # ---------------- phase 1 block worker ----------------
            # PSUM-reading copies only on vector/scalar (GPSIMD has no PSUM
            # access).  Blocks 0-3 split copies across both; blocks 4-7 put
            # them all on scalar so vector can run the Chebyshev recurrences.
            hview = h_d.ap().rearrange("(a p) f -> p a f", p=128)

            def phase1_block(blk, engines):
                ldb = wpool.tile([128, 8 * F], F32, tag="ldb", bufs=3)
                nc.sync.dma_start(ldb[:], hview[:, blk * 8 : (blk + 1) * 8, :])
                for g in range(2):
                    jt0 = blk * 8 + g * 4
                    trp2 = ppool.tile([128, 256], F32, tag="mix", bufs=6)
                    for k in range(2):
                        nc.tensor.transpose(
                            trp2[:, k * 128 : (k + 1) * 128],
                            ldb[:, (g * 4 + 2 * k) * F : (g * 4 + 2 * k + 2) * F],
                            ident32[:],
                        )
                    blk2 = jt0 // 2  # hT2 block index of jt0, jt0+1
                    cp(engines[jt0 // 4 % len(engines)],
                       hT2[:, blk2 * 128 : (blk2 + 2) * 128], trp2[:])
                    # per-tile-position psum tiles: mixing tile_positions as
                    # slice-writes of ONE psum tile wedges the PE
                    hp2 = [
                        ppool.tile([128, 2 * (F + 2)], F32, tag="mix", bufs=6,
                                   name=f"hp2_{jt0}_{i}")
                        for i in range(2)
                    ]
                    for k in range(4):
                        jt = jt0 + k
                        lo = (jt % 2) * F
                        dst = hp2[k % 2][:].rearrange(
                            "p (t c) -> p t c", c=F + 2)[:, k // 2, :]
                        nc.tensor.matmul(
                            dst,
                            hT2[lo : lo + F, (jt // 2) * 128 : (jt // 2 + 1) * 128],
                            waug16[lo : lo + F, :],
                            tile_position=(lo, 0),
                        )
                    for i in range(2):
                        h3 = hp2[i][:].rearrange("p (t c) -> p t c", c=F + 2)
                        eng = engines[(jt0 // 4 + i) % len(engines)]
                        cp(eng, Haug3[:, jt0 + i : jt0 + 4 : 2, 0:F],
                           h3[:, :, 0:F])
                        cp(eng, sd3[:, jt0 + i : jt0 + 4 : 2, :],
                           h3[:, :, F : F + 2])

            # ---------------- Chebyshev helpers ----------------
            def cheb(dst3, state3, src, n, scale_col, exp_engine):
                """dst3 [128, n, MB] f16, state3 [128, R, n] f32, src [128, n]
                strided f32 (x^); fills T_0..T_30 and e^x (col MB-1)."""
                exp_engine.activation(
                    dst3[:, :, MB - 1], src, AF.Exp,
                    scale=scal_sb[:, scale_col : scale_col + 1],
                )
                nc.vector.tensor_copy(dst3[:, :, 1], src)
                x2 = wpool.tile([128, n], F32, tag=f"x2_{n}", bufs=2)
                nc.vector.tensor_scalar_mul(x2[:], src, 2.0)
                nc.gpsimd.tensor_copy(state3[:, 1, :], src)
                for b in range(2, R):
                    tmp = wpool.tile([128, n], F32, tag=f"tmp_{n}", bufs=2)
                    nc.vector.tensor_mul(tmp[:], x2[:], state3[:, b - 1, :])
                    nc.vector.tensor_sub(state3[:, b, :], tmp[:], state3[:, b - 2, :])
                    if b % 2 == 1:  # write cols b-1, b together
                        eng = nc.gpsimd if (b // 2) % 2 == 0 else nc.scalar
                        cp(eng, dst3[:, :, b - 1 : b + 1],
                           state3[:, b - 1 : b + 1, :].rearrange(
                               "p b t -> p t b"))
                cp(nc.gpsimd, dst3[:, :, R - 1 : R],
                   state3[:, R - 1 : R, :].rearrange("p b t -> p t b"))

            # ---------------- emit program ----------------
            nc.gpsimd.memset(T3[:, 0, :], 1.0)
            nc.gpsimd.memset(S3[:, 0, :], 1.0)

            both = [nc.vector, nc.scalar]
            for blk in range(4):
                phase1_block(blk, both)
            # first-half d recurrence + i-side recurrence on vector while
            # scalar alone handles phase-1 copies of blocks 4-7
            cheb(TDaug3[:, 0:32, :], T3[:, :, 0:32], sd3[:, 0:32, 1],
                 32, 1, nc.scalar)
            for blk in range(4, 8):
                phase1_block(blk, [nc.scalar])
            cheb(TDaug3[:, 32:64, :], T3[:, :, 32:64], sd3[:, 32:64, 1],
                 32, 1, nc.scalar)
            cheb(Fi3, S3, sd3[:, 0:NTI, 0], NTI, 0, nc.scalar)

            # ---------------- B = TDaug^T @ Haug ----------------
            B_ps = ppool.tile([MB, MH], F32, tag="acc", bufs=1)
            for jt in range(NT):
                nc.tensor.matmul(
                    B_ps[:], TDaug3[:, jt, :], Haug3[:, jt, :],
                    start=(jt == 0), stop=(jt == NT - 1),
                )
            B16 = cpool.tile([MB, MH], F16)
            nc.scalar.copy(B16[:], B_ps[:])

            # BKaug4 = E^T @ B  (4x replicated, pre-scaled)
            bk_ps = ppool.tile([128, MH], F32, tag="mix", bufs=6)
            nc.tensor.matmul(bk_ps[:], e16[:], B16[:])
            nc.vector.tensor_copy(BKaug4[:], bk_ps[:])

            # ---------------- transpose Faug (4 i-tiles per go) ----------------
            for q in range(NTI // 4):
                ftp = ppool.tile([128, 128], F16, tag="mix", bufs=6)
                nc.tensor.transpose(
                    ftp[:], Fi3[:, 4 * q : 4 * q + 4, :], ident16[:]
                )
                cp(both[q % 2], F33[:, q, :], ftp[:])

            # ---------------- synthesis + epilogue ----------------
            for ch in range(NTI // 4):
                o2c = wpool.tile([128, 4 * F], F32, tag="o2c", bufs=2)
                for sub in range(4):
                    it = ch * 4 + sub
                    q, k = it // 4, it % 4
                    lo = 32 * k
                    ot_ps = ppool.tile([128, MH], F32, tag="mix", bufs=6)
                    nc.tensor.matmul(
                        ot_ps[:],
                        F33[lo : lo + 32, q, :],
                        BKaug4[lo : lo + 32, :],
                        tile_position=(lo, 0),
                    )
                    rec = wpool.tile([128, 1], F32, tag="rec", bufs=4)
                    nc.vector.reciprocal(rec[:], ot_ps[:, MH - 1 : MH])
                    o1 = wpool.tile([128, F], F32, tag="o1", bufs=4)
                    if sub % 2 == 0:
                        nc.scalar.mul(o1[:], ot_ps[:, 0:F], rec[:])
                    else:
                        nc.vector.tensor_scalar_mul(o1[:], ot_ps[:, 0:F], rec[:])
                    nc.gpsimd.tensor_add(
                        o2c[:, sub * F : (sub + 1) * F], o1[:], biasr[:]
                    )
                out_view = out_d.ap().rearrange("(a p) f -> p a f", p=128)
                nc.sync.dma_start(
                    out_view[:, ch * 4 : (ch + 1) * 4, :], o2c[:]
                )

    nc.compile()
    return nc


_NC_CACHE = None


def _get_nc():
    global _NC_CACHE
    if _NC_CACHE is None:
        _NC_CACHE = _build_kernel_module()
    return _NC_CACHE


def _cheb_fit_K(Ls, Ld, Ra, Rb, ngrid=96):
    """Chebyshev product coefficients of K(s,d) = exp(.2(s+d))-exp(s+d) (s+d<0)
    on [-Ls,Ls] x [-Ld,Ld].  Returns C [Ra, Rb]."""
    ks = np.cos((np.arange(ngrid) + 0.5) * np.pi / ngrid)
    S, D = np.meshgrid(ks * Ls, ks * Ld, indexing="ij")
    X = S + D
    K = np.where(X < 0, np.exp(ALPHA * X) - np.exp(X), 0.0)
    Ts = np.polynomial.chebyshev.chebvander(ks, Ra - 1)   # [ngrid, Ra]
    Td = np.polynomial.chebyshev.chebvander(ks, Rb - 1)
    C = (2.0 / ngrid) ** 2 * np.einsum("ij,ia,jb->ab", K, Ts, Td)
    C[0, :] *= 0.5
    C[:, 0] *= 0.5
    return C


def _make_in_maps(h, w, a_src, a_dst, bias):
    h = np.ascontiguousarray(np.asarray(h, dtype=np.float32))
    w = np.asarray(w, dtype=np.float32)
    a_src = np.asarray(a_src, dtype=np.float32)
    a_dst = np.asarray(a_dst, dtype=np.float32)
    bias = np.asarray(bias, dtype=np.float32)
    biasr = np.ascontiguousarray(np.broadcast_to(bias[None, :], (128, F)))

    in_maps = []
    for c in range(8):
        head, half = c // 2, c % 2
        ws = w[head] @ a_src[head][:, 0]          # [64]
        wd = w[head] @ a_dst[head][:, 0]
        s_all = h @ ws
        d_all = h @ wd
        Ls = float(np.abs(s_all).max()) * 1.02 + 1e-30
        Ld = float(np.abs(d_all).max()) * 1.02 + 1e-30
        C = _cheb_fit_K(Ls, Ld, R, R)
        waug = np.concatenate(
            [w[head], (ws / Ls)[:, None], (wd / Ld)[:, None]], axis=1
        ).astype(np.float32)
        # E[b, 32k+a] = scale*C[a, b] (a,b < R); E[R, 32k+R] = scale
        E = np.zeros((MB, 128), dtype=np.float32)
        for k in range(4):
            E[0:R, 32 * k : 32 * k + R] = ND_SCALE * C.T
            E[R, 32 * k + R] = ND_SCALE
        scal = np.broadcast_to(
            np.array([Ls, Ld], dtype=np.float32)[None, :], (128, 2)
        )
        # rotate h so this core's own half sits in j-tiles 0..31
        h_rot = h if half == 0 else np.concatenate([h[HALF:], h[:HALF]], axis=0)
        in_maps.append(
            {
                "hfull": np.ascontiguousarray(h_rot),
                "waug": np.ascontiguousarray(waug),
                "emat": E,
                "scal": np.ascontiguousarray(scal),
                "biasr": biasr,
            }
        )
    return in_maps


def _run(h, w, a_src, a_dst, bias, trace=False, **trace_kwargs):
    nc = _get_nc()
    in_maps = _make_in_maps(h, w, a_src, a_dst, bias)
    res = run_bass_kernel_spmd(
        nc, in_maps, core_ids=list(range(8)), trace=trace, **trace_kwargs
    )
    out = np.zeros((BS, NH * F), dtype=np.float32)
    for c in range(8):
        head, half = c // 2, c % 2
        out[half * HALF : (half + 1) * HALF, head * F : (head + 1) * F] = res.results[
            c
        ]["out"]
    return out, res


def kernel(h, w, a_src, a_dst, bias):
    out, _ = _run(h, w, a_src, a_dst, bias, trace=False)
    return out


# revision 21
# speedup vs baseline: 4.5066x; 1.0777x over previous
"""Multi-head graph-attention (GAT) kernel for Trainium2, 8 NeuronCores.

Reference computation (per head):
    h_prime = h @ w[head]                       # [8192, 64]
    s = h_prime @ a_src[head],  d = h_prime @ a_dst[head]
    attn = softmax_j(leaky_relu(s_i + d_j, 0.2))
    out  = attn @ h_prime + bias                # -> [8192, 4*64]

Low-rank reformulation (no O(n^2) work on device):
    W[i,j] = exp(lrelu(s_i + d_j)) = e^{s_i} e^{d_j} + K(s_i, d_j)
  where K(s,d) = exp(0.2(s+d)) - exp(s+d) for s+d < 0, else 0, is a bounded
  continuous function on the (s,d) rectangle covered by the data.  K is
  approximated by a rank-R Chebyshev product expansion (R=31):
    K(s,d) ~= sum_{a,b} c_ab T_a(s/Ls) T_b(d/Ld)
  fitted ON THE HOST per head (coefficients + ranges are runtime inputs).
  With Haug = [h' | 0 0 | 1] and TDaug = [T_0..T_30 | e^d] (j on partitions):
    B      = TDaug^T @ Haug                      # [32, 67]   (PE, O(n R))
    BKaug4 = E^T @ B                             # [128, 67]  one matmul; E is
             host-built so rows 32k+a give sum_b c_ab B_Tb + e^d row at 32k+31,
             replicated for k=0..3 and pre-scaled by 2^-8
    out^T  = Faug-tile^T @ BKaug4[32k:32k+32]    # [128 i, 67] per i-tile,
  where Faug = [T_0..T_30 | e^s] of the i side, transposed 4 tiles at a time
  (partition bands 0/32/64/96).  Row 66 of out^T is the softmax denominator;
  the epilogue divides, adds bias and stores.  Max rel err vs the reference is
  ~1e-3 (fp16 operands, fp32 psum accumulation), inside the 2e-2 gate.

Sharding: 8 cores = 4 heads x 2 row-halves (head parallel + bs row shard).
Each core gets full h (rotated so its own half is first) plus per-head
host-fitted constants; no collectives.
"""

import numpy as np

import concourse.bass as bass
import concourse.tile as tile
from concourse import bacc, mybir
from concourse.bass_utils import run_bass_kernel_spmd
from concourse.masks import make_identity

F32 = mybir.dt.float32
F16 = mybir.dt.float16
AF = mybir.ActivationFunctionType

BS = 8192          # nodes
F = 64             # f_in == f_out
NH = 4             # heads
HALF = BS // 2     # rows per core (row-half)
NT = BS // 128     # 64 j tiles
NTI = HALF // 128  # 32 i tiles
R = 31             # Chebyshev rank
MB = R + 1         # TDaug / Faug columns: T_0..T_{R-1}, e^d (resp e^s)
MH = 67            # Haug columns: h'(64), 2 zero, ones
ALPHA = 0.2
ND_SCALE = 2.0 ** -8   # folded into E so num/den stay small


def _build_kernel_module(has_bias):
    nc = bacc.Bacc("TRN2", target_bir_lowering=False, debug=False)

    h_d = nc.dram_tensor("hfull", [BS, F], F32, kind="ExternalInput")
    # waug: [w | (w@a_src)/Ls | (w@a_dst)/Ld]
    waug_d = nc.dram_tensor("waug", [F, F + 2], F32, kind="ExternalInput")
    # E: [MB, 128] expansion matrix (Cheb coeffs -> 4x replicated BKaug rows)
    e_d = nc.dram_tensor("emat", [MB, 128], F32, kind="ExternalInput")
    # scal: [128, 2] (Ls, Ld) replicated across partitions
    scal_d = nc.dram_tensor("scal", [128, 2], F32, kind="ExternalInput")
    if has_bias:
        # bias replicated across partitions
        biasr_d = nc.dram_tensor("biasr", [128, F], F32, kind="ExternalInput")
    out_d = nc.dram_tensor("out", [HALF, F], F32, kind="ExternalOutput")

    with tile.TileContext(nc) as tc:
        with (
            tc.tile_pool(name="const", bufs=1) as cpool,
            tc.tile_pool(name="work", bufs=3) as wpool,
            tc.tile_pool(name="psum", bufs=2, space="PSUM") as ppool,
        ):
            # first h block DMA goes on the queue BEFORE the const DMAs so
            # the PE can start transposing as early as possible
            hview = h_d.ap().rearrange("(a p) f -> p a f", p=128)
            ldb0 = wpool.tile([128, 8 * F], F32, tag="ldb", bufs=4)
            nc.sync.dma_start(ldb0[:], hview[:, 0:8, :])

            # ---------------- constants ----------------
            ident32 = cpool.tile([128, 128], F32)
            make_identity(nc, ident32[:])
            ident16 = cpool.tile([128, 128], F16)
            nc.gpsimd.tensor_copy(ident16[:], ident32[:])

            # waug on partitions 0-63 AND 64-127 (for row-band hp matmuls)
            waug_sb = cpool.tile([128, F + 2], F32)
            nc.sync.dma_start(waug_sb[0:F, :], waug_d.ap())
            nc.sync.dma_start(waug_sb[F:128, :], waug_d.ap())
            waug16 = cpool.tile([128, F + 2], F16)
            nc.gpsimd.tensor_copy(waug16[:], waug_sb[:])

            e_sb = cpool.tile([MB, 128], F32)
            nc.sync.dma_start(e_sb[:], e_d.ap())
            e16 = cpool.tile([MB, 128], F16)
            nc.gpsimd.tensor_copy(e16[:], e_sb[:])

            scal_sb = cpool.tile([128, 2], F32)
            nc.sync.dma_start(scal_sb[:], scal_d.ap())
            if has_bias:
                biasr = cpool.tile([128, F], F32)
                nc.sync.dma_start(biasr[:], biasr_d.ap())

            # ---------------- big SBUF tensors ----------------
            # h^T stored as 2-tile blocks: block t holds j-tiles (2t, 2t+1) on
            # partition halves 0-63 / 64-127
            hT2 = cpool.tile([128, (NT // 2) * 128], F16)
            Haug = cpool.tile([128, NT * MH], F16)      # [h' | 0 0 | 1]
            Haug3 = Haug[:].rearrange("p (t c) -> p t c", c=MH)
            sd = cpool.tile([128, NT * 2], F32)         # [s^, d^] per j tile
            sd3 = sd[:].rearrange("p (t c) -> p t c", c=2)
            TDaug = cpool.tile([128, NT * MB], F16)     # [T_0..T_30 | e^d]
            TDaug3 = TDaug[:].rearrange("p (t c) -> p t c", c=MB)
            T_all = cpool.tile([128, R * NT], F32)      # fp32 recurrence state
            T3 = T_all[:].rearrange("p (b t) -> p b t", t=NT)
            Fi = cpool.tile([128, NTI * MB], F16)       # i-side [T_a | e^s]
            Fi3 = Fi[:].rearrange("p (t c) -> p t c", c=MB)
            S_all = cpool.tile([128, R * NTI], F32)
            S3 = S_all[:].rearrange("p (b t) -> p b t", t=NTI)
            # Faug^T: block q holds i-tiles 4q..4q+3 on partition bands 32k
            F3t = cpool.tile([128, (NTI // 4) * 128], F16)
            F33 = F3t[:].rearrange("p (t c) -> p t c", c=128)
            BKaug4 = cpool.tile([128, MH], F16)

            nc.gpsimd.memset(Haug3[:, :, F : F + 2], 0.0)
            nc.gpsimd.memset(Haug3[:, :, MH - 1], 1.0)
            nc.gpsimd.memset(TDaug3[:, :, 0], 1.0)
            nc.gpsimd.memset(Fi3[:, :, 0], 1.0)

            def cp(e, out, in_):
                (e.copy if e is nc.scalar else e.tensor_copy)(out, in_)

            # ---------------- phase 1 block worker ----------------
            # PSUM-reading copies only on vector/scalar (GPSIMD has no PSUM
            # access).  Blocks 0-3 split copies across both; blocks 4-7 put
            # them all on scalar so vector can run the Chebyshev recurrences.
            def phase1_block(blk, engines, ldb=None):
                if ldb is None:
                    ldb = wpool.tile([128, 8 * F], F32, tag="ldb", bufs=4)
                    nc.sync.dma_start(
                        ldb[:], hview[:, blk * 8 : (blk + 1) * 8, :]
                    )
                for g in range(2):
                    jt0 = blk * 8 + g * 4
                    trp2 = ppool.tile([128, 256], F32, tag="mix", bufs=6)
                    for k in range(2):
                        nc.tensor.transpose(
                            trp2[:, k * 128 : (k + 1) * 128],
                            ldb[:, (g * 4 + 2 * k) * F : (g * 4 + 2 * k + 2) * F],
                            ident32[:],
                        )
                    blk2 = jt0 // 2  # hT2 block index of jt0, jt0+1
                    cp(engines[jt0 // 4 % len(engines)],
                       hT2[:, blk2 * 128 : (blk2 + 2) * 128], trp2[:])
                    # per-tile-position psum tiles: mixing tile_positions as
                    # slice-writes of ONE psum tile wedges the PE
                    hp2 = [
                        ppool.tile([128, 2 * (F + 2)], F32, tag="mix", bufs=6,
                                   name=f"hp2_{jt0}_{i}")
                        for i in range(2)
                    ]
                    for k in range(4):
                        jt = jt0 + k
                        lo = (jt % 2) * F
                        dst = hp2[k % 2][:].rearrange(
                            "p (t c) -> p t c", c=F + 2)[:, k // 2, :]
                        nc.tensor.matmul(
                            dst,
                            hT2[lo : lo + F, (jt // 2) * 128 : (jt // 2 + 1) * 128],
                            waug16[lo : lo + F, :],
                            tile_position=(lo, 0),
                        )
                    for i in range(2):
                        h3 = hp2[i][:].rearrange("p (t c) -> p t c", c=F + 2)
                        eng = engines[(jt0 // 4 + i) % len(engines)]
                        cp(eng, Haug3[:, jt0 + i : jt0 + 4 : 2, 0:F],
                           h3[:, :, 0:F])
                        cp(eng, sd3[:, jt0 + i : jt0 + 4 : 2, :],
                           h3[:, :, F : F + 2])

            # ---------------- Chebyshev helpers ----------------
            def cheb(dst3, state3, src, n, scale_col, exp_engine):
                """dst3 [128, n, MB] f16, state3 [128, R, n] f32, src [128, n]
                strided f32 (x^); fills T_0..T_30 and e^x (col MB-1)."""
                exp_engine.activation(
                    dst3[:, :, MB - 1], src, AF.Exp,
                    scale=scal_sb[:, scale_col : scale_col + 1],
                )
                nc.vector.tensor_copy(dst3[:, :, 1], src)
                x2 = wpool.tile([128, n], F32, tag=f"x2_{n}", bufs=2)
                nc.vector.tensor_scalar_mul(x2[:], src, 2.0)
                nc.gpsimd.tensor_copy(state3[:, 1, :], src)
                for b in range(2, R):
                    tmp = wpool.tile([128, n], F32, tag=f"tmp_{n}", bufs=2)
                    nc.vector.tensor_mul(tmp[:], x2[:], state3[:, b - 1, :])
                    nc.vector.tensor_sub(state3[:, b, :], tmp[:], state3[:, b - 2, :])
                    if b % 2 == 1:  # write cols b-1, b together
                        eng = nc.gpsimd if (b // 2) % 2 == 0 else nc.scalar
                        cp(eng, dst3[:, :, b - 1 : b + 1],
                           state3[:, b - 1 : b + 1, :].rearrange(
                               "p b t -> p t b"))
                cp(nc.gpsimd, dst3[:, :, R - 1 : R],
                   state3[:, R - 1 : R, :].rearrange("p b t -> p t b"))

            # ---------------- emit program ----------------
            nc.gpsimd.memset(T3[:, 0, :], 1.0)
            nc.gpsimd.memset(S3[:, 0, :], 1.0)

            both = [nc.vector, nc.scalar]
            phase1_block(0, both, ldb=ldb0)
            for blk in range(1, 4):
                phase1_block(blk, both)
            # vector runs the recurrences (d half 0, then i side, then d half
            # 1) while scalar alone handles phase-1 copies of blocks 4-7
            cheb(TDaug3[:, 0:32, :], T3[:, :, 0:32], sd3[:, 0:32, 1],
                 32, 1, nc.scalar)
            cheb(Fi3, S3, sd3[:, 0:NTI, 0], NTI, 0, nc.scalar)
            for blk in range(4, 8):
                phase1_block(blk, [nc.scalar])
            cheb(TDaug3[:, 32:64, :], T3[:, :, 32:64], sd3[:, 32:64, 1],
                 32, 1, nc.scalar)

            # ---------------- B = TDaug^T @ Haug ----------------
            # split in halves so the Faug transposes can fill the PE while
            # the second-half d recurrence finishes
            B_ps = ppool.tile([MB, MH], F32, tag="acc", bufs=1)
            for jt in range(32):
                nc.tensor.matmul(
                    B_ps[:], TDaug3[:, jt, :], Haug3[:, jt, :],
                    start=(jt == 0), stop=False,
                )

            # transpose Faug (4 i-tiles per go)
            for q in range(NTI // 4):
                ftp = ppool.tile([128, 128], F16, tag="mix", bufs=6)
                nc.tensor.transpose(
                    ftp[:], Fi3[:, 4 * q : 4 * q + 4, :], ident16[:]
                )
                cp(both[q % 2], F33[:, q, :], ftp[:])

            for jt in range(32, NT):
                nc.tensor.matmul(
                    B_ps[:], TDaug3[:, jt, :], Haug3[:, jt, :],
                    start=False, stop=(jt == NT - 1),
                )
            B16 = cpool.tile([MB, MH], F16)
            nc.scalar.copy(B16[:], B_ps[:])

            # BKaug4 = E^T @ B  (4x replicated, pre-scaled)
            bk_ps = ppool.tile([128, MH], F32, tag="mix", bufs=6)
            nc.tensor.matmul(bk_ps[:], e16[:], B16[:])
            nc.vector.tensor_copy(BKaug4[:], bk_ps[:])

            # ---------------- synthesis + epilogue ----------------
            out_view = out_d.ap().rearrange("(a p) f -> p a f", p=128)
            for ch in range(NTI // 4):
                o1c = wpool.tile([128, 4 * F], F32, tag="o1c", bufs=2)
                for sub in range(4):
                    it = ch * 4 + sub
                    q, k = it // 4, it % 4
                    lo = 32 * k
                    ot_ps = ppool.tile([128, MH], F32, tag="mix", bufs=6)
                    nc.tensor.matmul(
                        ot_ps[:],
                        F33[lo : lo + 32, q, :],
                        BKaug4[lo : lo + 32, :],
                        tile_position=(lo, 0),
                    )
                    rec = wpool.tile([128, 1], F32, tag="rec", bufs=4)
                    nc.vector.reciprocal(rec[:], ot_ps[:, MH - 1 : MH])
                    if has_bias:
                        o1 = wpool.tile([128, F], F32, tag="o1", bufs=4)[:]
                    else:
                        o1 = o1c[:, sub * F : (sub + 1) * F]
                    if sub % 2 == 0:
                        nc.scalar.mul(o1, ot_ps[:, 0:F], rec[:])
                    else:
                        nc.vector.tensor_scalar_mul(o1, ot_ps[:, 0:F], rec[:])
                    if has_bias:
                        nc.gpsimd.tensor_add(
                            o1c[:, sub * F : (sub + 1) * F], o1, biasr[:]
                        )
                nc.sync.dma_start(
                    out_view[:, ch * 4 : (ch + 1) * 4, :], o1c[:]
                )

    nc.compile()
    return nc


_NC_CACHE = {}


def _get_nc(has_bias):
    if has_bias not in _NC_CACHE:
        _NC_CACHE[has_bias] = _build_kernel_module(has_bias)
    return _NC_CACHE[has_bias]


def _cheb_fit_K(Ls, Ld, Ra, Rb, ngrid=96):
    """Chebyshev product coefficients of K(s,d) = exp(.2(s+d))-exp(s+d) (s+d<0)
    on [-Ls,Ls] x [-Ld,Ld].  Returns C [Ra, Rb]."""
    ks = np.cos((np.arange(ngrid) + 0.5) * np.pi / ngrid)
    S, D = np.meshgrid(ks * Ls, ks * Ld, indexing="ij")
    X = S + D
    K = np.where(X < 0, np.exp(ALPHA * X) - np.exp(X), 0.0)
    Ts = np.polynomial.chebyshev.chebvander(ks, Ra - 1)   # [ngrid, Ra]
    Td = np.polynomial.chebyshev.chebvander(ks, Rb - 1)
    C = (2.0 / ngrid) ** 2 * np.einsum("ij,ia,jb->ab", K, Ts, Td)
    C[0, :] *= 0.5
    C[:, 0] *= 0.5
    return C


def _make_in_maps(h, w, a_src, a_dst, bias):
    h = np.ascontiguousarray(np.asarray(h, dtype=np.float32))
    w = np.asarray(w, dtype=np.float32)
    a_src = np.asarray(a_src, dtype=np.float32)
    a_dst = np.asarray(a_dst, dtype=np.float32)
    bias = np.asarray(bias, dtype=np.float32)
    has_bias = bool(np.any(bias != 0.0))
    biasr = np.ascontiguousarray(np.broadcast_to(bias[None, :], (128, F)))

    in_maps = []
    for c in range(8):
        head, half = c // 2, c % 2
        ws = w[head] @ a_src[head][:, 0]          # [64]
        wd = w[head] @ a_dst[head][:, 0]
        s_all = h @ ws
        d_all = h @ wd
        Ls = float(np.abs(s_all).max()) * 1.02 + 1e-30
        Ld = float(np.abs(d_all).max()) * 1.02 + 1e-30
        C = _cheb_fit_K(Ls, Ld, R, R)
        waug = np.concatenate(
            [w[head], (ws / Ls)[:, None], (wd / Ld)[:, None]], axis=1
        ).astype(np.float32)
        # E[b, 32k+a] = scale*C[a, b] (a,b < R); E[R, 32k+R] = scale
        E = np.zeros((MB, 128), dtype=np.float32)
        for k in range(4):
            E[0:R, 32 * k : 32 * k + R] = ND_SCALE * C.T
            E[R, 32 * k + R] = ND_SCALE
        scal = np.broadcast_to(
            np.array([Ls, Ld], dtype=np.float32)[None, :], (128, 2)
        )
        # rotate h so this core's own half sits in j-tiles 0..31
        h_rot = h if half == 0 else np.concatenate([h[HALF:], h[:HALF]], axis=0)
        m = {
            "hfull": np.ascontiguousarray(h_rot),
            "waug": np.ascontiguousarray(waug),
            "emat": E,
            "scal": np.ascontiguousarray(scal),
        }
        if has_bias:
            m["biasr"] = biasr
        in_maps.append(m)
    return has_bias, in_maps


def _run(h, w, a_src, a_dst, bias, trace=False, **trace_kwargs):
    has_bias, in_maps = _make_in_maps(h, w, a_src, a_dst, bias)
    nc = _get_nc(has_bias)
    res = run_bass_kernel_spmd(
        nc, in_maps, core_ids=list(range(8)), trace=trace, **trace_kwargs
    )
    out = np.zeros((BS, NH * F), dtype=np.float32)
    for c in range(8):
        head, half = c // 2, c % 2
        out[half * HALF : (half + 1) * HALF, head * F : (head + 1) * F] = res.results[
            c
        ]["out"]
    return out, res


def kernel(h, w, a_src, a_dst, bias):
    out, _ = _run(h, w, a_src, a_dst, bias, trace=False)
    return out


# revision 26
# speedup vs baseline: 5.5497x; 1.2315x over previous
"""Multi-head graph-attention (GAT) kernel for Trainium2, 8 NeuronCores.

Reference computation (per head):
    h_prime = h @ w[head]                       # [8192, 64]
    s = h_prime @ a_src[head],  d = h_prime @ a_dst[head]
    attn = softmax_j(leaky_relu(s_i + d_j, 0.2))
    out  = attn @ h_prime + bias                # -> [8192, 4*64]

Low-rank reformulation (no O(n^2) work on device):
    W[i,j] = exp(lrelu(s_i + d_j)) = e^{s_i} e^{d_j} + K(s_i, d_j)
  where K(s,d) = exp(0.2(s+d)) - exp(s+d) for s+d < 0, else 0, is a bounded
  continuous function on the (s,d) rectangle covered by the data.  K is
  approximated by a rank-31 product expansion fitted ON THE HOST per head
  (least squares; coefficients and ranges become runtime inputs):
    K(s,d) ~= sum_{a,b} C[a,b] f_a(s/Ls) g_b(d/Ld)
  f_a = Chebyshev T_a (serial DVE recurrence on the small i side),
  g_b = Fourier {1, sin(k w x), cos(k w x)} (30 independent, chain-free
  Sin activations on the scalar engine for the large j side, issued in two
  row-halves so they overlap the h'-building phase).
  With Haug = [h' | 0 0 | 1] and TDaug = [g_0..g_30 | e^d] (j on partitions):
    B      = TDaug^T @ Haug                      # [32, 67]   (PE, O(n R))
    BKaug4 = E^T @ B                             # [128, 67]  one matmul; E is
             host-built: rows 32k+a = sum_b C[a,b] B_gb + e^d row at 32k+31,
             replicated for k=0..3 and pre-scaled by 2^-8
    out^T  = Faug-tile^T @ BKaug4[32k:32k+32]    # [128 i, 67] per i-tile
  where Faug = [T_0..T_30 | e^s], transposed 4 i-tiles at a time (partition
  bands 0/32/64/96 + matching tile_position).  Row 66 of out^T is the softmax
  denominator; the epilogue divides (+bias) and stores.  Max rel err vs the
  reference is ~7e-4 (fp16 operands, fp32 psum accumulation): 25x inside the
  2e-2 gate.

Sharding: 8 cores = 4 heads x 2 row-halves (head parallel + bs row shard).
Each core gets full h (rotated so its own half is first) plus per-head
host-fitted constants; no collectives.
"""

import numpy as np

import concourse.bass as bass
import concourse.tile as tile
from concourse import bacc, mybir
from concourse.bass_utils import run_bass_kernel_spmd
from concourse.masks import make_identity

F32 = mybir.dt.float32
F16 = mybir.dt.float16
AF = mybir.ActivationFunctionType

BS = 8192          # nodes
F = 64             # f_in == f_out
NH = 4             # heads
HALF = BS // 2     # rows per core (row-half)
NT = BS // 128     # 64 j tiles
NTI = HALF // 128  # 32 i tiles
R = 31             # expansion rank per side
MB = R + 1         # TDaug / Faug columns (basis + e^x)
MH = 67            # Haug columns: h'(64), 2 zero, ones
BETA = 14.0        # tanh soft-step sharpness (d-side basis)
SHIFT_EXT = 1.1    # shift extent of the 30 tanh steps
ALPHA = 0.2
ND_SCALE = 2.0 ** -8   # folded into E so num/den stay small


def _build_kernel_module(has_bias):
    nc = bacc.Bacc("TRN2", target_bir_lowering=False, debug=False)

    h_d = nc.dram_tensor("hfull", [BS, F], F32, kind="ExternalInput")
    # waug: [w | (w@a_src)/Ls | (w@a_dst)/Ld]
    waug_d = nc.dram_tensor("waug", [F, F + 2], F32, kind="ExternalInput")
    # E: [MB, 128] expansion matrix (fit coeffs -> 4x replicated BKaug rows)
    e_d = nc.dram_tensor("emat", [MB, 128], F32, kind="ExternalInput")
    # scal: [128, 2] (Ls, Ld) replicated across partitions
    scal_d = nc.dram_tensor("scal", [128, 2], F32, kind="ExternalInput")
    if has_bias:
        biasr_d = nc.dram_tensor("biasr", [128, F], F32, kind="ExternalInput")
    out_d = nc.dram_tensor("out", [HALF, F], F32, kind="ExternalOutput")

    with tile.TileContext(nc) as tc:
        with (
            tc.tile_pool(name="const", bufs=1) as cpool,
            tc.tile_pool(name="work", bufs=3) as wpool,
            tc.tile_pool(name="psum", bufs=2, space="PSUM") as ppool,
        ):
            # first h block DMA goes on the queue BEFORE the const DMAs so
            # the PE can start transposing as early as possible
            hview = h_d.ap().rearrange("(a p) f -> p a f", p=128)
            ldb0 = wpool.tile([128, 8 * F], F32, tag="ldb", bufs=4)
            nc.sync.dma_start(ldb0[:], hview[:, 0:8, :])

            # ---------------- constants ----------------
            ident32 = cpool.tile([128, 128], F32)
            make_identity(nc, ident32[:])
            ident16 = cpool.tile([128, 128], F16)
            nc.gpsimd.tensor_copy(ident16[:], ident32[:])

            # waug on partitions 0-63 AND 64-127 (for row-band hp matmuls)
            waug_sb = cpool.tile([128, F + 2], F32)
            nc.sync.dma_start(waug_sb[0:F, :], waug_d.ap())
            nc.sync.dma_start(waug_sb[F:128, :], waug_d.ap())
            waug16 = cpool.tile([128, F + 2], F16)
            nc.gpsimd.tensor_copy(waug16[:], waug_sb[:])

            e_sb = cpool.tile([MB, 128], F32)
            nc.sync.dma_start(e_sb[:], e_d.ap())
            e16 = cpool.tile([MB, 128], F16)
            nc.gpsimd.tensor_copy(e16[:], e_sb[:])

            scal_sb = cpool.tile([128, 2], F32)
            nc.sync.dma_start(scal_sb[:], scal_d.ap())
            if has_bias:
                biasr = cpool.tile([128, F], F32)
                nc.sync.dma_start(biasr[:], biasr_d.ap())

            # ---------------- big SBUF tensors ----------------
            # h^T in 2-tile blocks: block t holds j-tiles (2t, 2t+1) on
            # partition halves 0-63 / 64-127
            hT2 = cpool.tile([128, (NT // 2) * 128], F16)
            Haug = cpool.tile([128, NT * MH], F16)      # [h' | 0 0 | 1]
            Haug3 = Haug[:].rearrange("p (t c) -> p t c", c=MH)
            sd = cpool.tile([128, NT * 2], F32)         # [s^, d^] per j tile
            sd3 = sd[:].rearrange("p (t c) -> p t c", c=2)
            TDaug = cpool.tile([128, NT * MB], F16)     # [fourier(d^) | e^d]
            TDaug3 = TDaug[:].rearrange("p (t c) -> p t c", c=MB)
            Fi = cpool.tile([128, NTI * MB], F16)       # [cheb(s^) | e^s]
            Fi3 = Fi[:].rearrange("p (t c) -> p t c", c=MB)
            S_all = cpool.tile([128, R * NTI], F32)     # cheb fp32 state
            S3 = S_all[:].rearrange("p (b t) -> p b t", t=NTI)
            # Faug^T: block q holds i-tiles 4q..4q+3 on partition bands 32k
            F3t = cpool.tile([128, (NTI // 4) * 128], F16)
            F33 = F3t[:].rearrange("p (t c) -> p t c", c=128)
            BKaug4 = cpool.tile([128, MH], F16)

            nc.gpsimd.memset(Haug3[:, :, F : F + 2], 0.0)
            nc.gpsimd.memset(Haug3[:, :, MH - 1], 1.0)
            nc.gpsimd.memset(TDaug3[:, :, 0], 1.0)
            nc.gpsimd.memset(Fi3[:, :, 0], 1.0)
            nc.gpsimd.memset(S3[:, 0, :], 1.0)

            def cp(e, out, in_):
                (e.copy if e is nc.scalar else e.tensor_copy)(out, in_)

            # ---------------- phase 1 block worker ----------------
            # PSUM-reading copies only on vector/scalar (GPSIMD has no PSUM
            # access).  eng_ht drains the transposes, eng_hs drains h'.
            def phase1_block(blk, eng_ht, eng_hs, ldb=None):
                if ldb is None:
                    ldb = wpool.tile([128, 8 * F], F32, tag="ldb", bufs=4)
                    nc.sync.dma_start(
                        ldb[:], hview[:, blk * 8 : (blk + 1) * 8, :]
                    )
                ldb16 = wpool.tile([128, 8 * F], F16, tag="ldb16", bufs=4)
                nc.gpsimd.tensor_copy(ldb16[:], ldb[:])
                for g in range(2):
                    jt0 = blk * 8 + g * 4
                    trp2 = ppool.tile([128, 256], F16, tag="mix", bufs=6)
                    for k in range(2):
                        nc.tensor.transpose(
                            trp2[:, k * 128 : (k + 1) * 128],
                            ldb16[:, (g * 4 + 2 * k) * F : (g * 4 + 2 * k + 2) * F],
                            ident16[:],
                        )
                    blk2 = jt0 // 2
                    cp(eng_ht[g], hT2[:, blk2 * 128 : (blk2 + 2) * 128], trp2[:])
                    # per-tile-position psum tiles: mixing tile_positions as
                    # slice-writes of ONE psum tile wedges the PE
                    hp2 = [
                        ppool.tile([128, 2 * (F + 2)], F32, tag="mix", bufs=6,
                                   name=f"hp2_{jt0}_{i}")
                        for i in range(2)
                    ]
                    for k in range(4):
                        jt = jt0 + k
                        lo = (jt % 2) * F
                        dst = hp2[k % 2][:].rearrange(
                            "p (t c) -> p t c", c=F + 2)[:, k // 2, :]
                        nc.tensor.matmul(
                            dst,
                            hT2[lo : lo + F, (jt // 2) * 128 : (jt // 2 + 1) * 128],
                            waug16[lo : lo + F, :],
                            tile_position=(lo, 0),
                        )
                    for i in range(2):
                        h3 = hp2[i][:].rearrange("p (t c) -> p t c", c=F + 2)
                        cp(eng_hs[(g + i) % 2],
                           Haug3[:, jt0 + i : jt0 + 4 : 2, 0:F], h3[:, :, 0:F])
                        cp(eng_hs[(g + i + 1) % 2],
                           sd3[:, jt0 + i : jt0 + 4 : 2, :], h3[:, :, F : F + 2])

            # ---------------- d side: tanh soft-step basis via scalar ACTs --
            # g_b(x) = tanh(BETA * (x - t_b)): one chain-free ACT per column
            shifts = np.linspace(-SHIFT_EXT, SHIFT_EXT, R - 1)
            tb_bias = cpool.tile([128, R - 1], F32)
            for b, t in enumerate(shifts):
                nc.gpsimd.memset(tb_bias[:, b : b + 1], float(-BETA * t))

            def d_tanh(lo, hi):
                dv = sd3[:, lo:hi, 1]
                nc.scalar.activation(
                    TDaug3[:, lo:hi, MB - 1], dv, AF.Exp, scale=scal_sb[:, 1:2]
                )
                for b in range(R - 1):
                    nc.scalar.activation(
                        TDaug3[:, lo:hi, 1 + b], dv, AF.Tanh,
                        scale=BETA, bias=tb_bias[:, b : b + 1],
                    )

            # ---------------- s side: Chebyshev recurrence on DVE ----------
            s_view = sd3[:, 0:NTI, 0]
            x2s = cpool.tile([128, NTI], F32)

            def s_cheb_setup():
                nc.scalar.activation(
                    Fi3[:, :, MB - 1], s_view, AF.Exp, scale=scal_sb[:, 0:1]
                )
                nc.gpsimd.tensor_copy(Fi3[:, :, 1], s_view)
                nc.vector.tensor_copy(S3[:, 1, :], s_view)
                nc.vector.tensor_scalar_mul(x2s[:], s_view, 2.0)

            def s_cheb_chunk(b0, b1):
                for b in range(b0, b1):
                    tmp = wpool.tile([128, NTI], F32, tag="tmps", bufs=2)
                    nc.vector.tensor_mul(tmp[:], x2s[:], S3[:, b - 1, :])
                    nc.vector.tensor_sub(S3[:, b, :], tmp[:], S3[:, b - 2, :])
                    if b % 2 == 0:
                        nc.gpsimd.tensor_copy(
                            Fi3[:, :, b - 1 : b + 1],
                            S3[:, b - 1 : b + 1, :].rearrange("p b t -> p t b"),
                        )

            # ---------------- emit program ----------------
            phase1_block(0, [nc.vector, nc.scalar], [nc.scalar, nc.vector],
                         ldb=ldb0)
            for blk in range(1, 4):
                et = [nc.vector, nc.scalar] if blk % 2 else [nc.scalar, nc.vector]
                phase1_block(blk, et, et[::-1])
            s_cheb_setup()
            d_tanh(0, 32)               # scalar, overlaps blocks 4-7
            s_chunks = [(2, 9), (9, 16), (16, 23), (23, R)]
            for blk in range(4, 8):
                phase1_block(blk, [nc.vector, nc.vector], [nc.scalar, nc.scalar])
                s_cheb_chunk(*s_chunks[blk - 4])
            d_tanh(32, 64)              # scalar, after block 7's sd lands

            # ---------------- B = TDaug^T @ Haug ----------------
            # split in halves so the Faug transposes can fill the PE while
            # the second-half d basis finishes
            B_ps = ppool.tile([MB, MH], F32, tag="acc", bufs=1)
            for jt in range(32):
                nc.tensor.matmul(
                    B_ps[:], TDaug3[:, jt, :], Haug3[:, jt, :],
                    start=(jt == 0), stop=False,
                )

            # transpose Faug (4 i-tiles per go)
            for q in range(NTI // 4):
                ftp = ppool.tile([128, 128], F16, tag="mix", bufs=6)
                nc.tensor.transpose(
                    ftp[:], Fi3[:, 4 * q : 4 * q + 4, :], ident16[:]
                )
                cp((nc.vector, nc.scalar)[q % 2], F33[:, q, :], ftp[:])

            for jt in range(32, NT):
                nc.tensor.matmul(
                    B_ps[:], TDaug3[:, jt, :], Haug3[:, jt, :],
                    start=False, stop=(jt == NT - 1),
                )
            B16 = cpool.tile([MB, MH], F16)
            nc.scalar.copy(B16[:], B_ps[:])

            # BKaug4 = E^T @ B  (4x replicated, pre-scaled)
            bk_ps = ppool.tile([128, MH], F32, tag="mix", bufs=6)
            nc.tensor.matmul(bk_ps[:], e16[:], B16[:])
            nc.vector.tensor_copy(BKaug4[:], bk_ps[:])

            # ---------------- synthesis + epilogue ----------------
            out_view = out_d.ap().rearrange("(a p) f -> p a f", p=128)
            for ch in range(NTI // 4):
                o1c = wpool.tile([128, 4 * F], F32, tag="o1c", bufs=2)
                for sub in range(4):
                    it = ch * 4 + sub
                    q, k = it // 4, it % 4
                    lo = 32 * k
                    ot_ps = ppool.tile([128, MH], F32, tag="mix", bufs=6)
                    nc.tensor.matmul(
                        ot_ps[:],
                        F33[lo : lo + 32, q, :],
                        BKaug4[lo : lo + 32, :],
                        tile_position=(lo, 0),
                    )
                    rec = wpool.tile([128, 1], F32, tag="rec", bufs=4)
                    nc.vector.reciprocal(rec[:], ot_ps[:, MH - 1 : MH])
                    if has_bias:
                        o1 = wpool.tile([128, F], F32, tag="o1", bufs=4)[:]
                    else:
                        o1 = o1c[:, sub * F : (sub + 1) * F]
                    if sub % 2 == 0:
                        nc.scalar.mul(o1, ot_ps[:, 0:F], rec[:])
                    else:
                        nc.vector.tensor_scalar_mul(o1, ot_ps[:, 0:F], rec[:])
                    if has_bias:
                        nc.gpsimd.tensor_add(
                            o1c[:, sub * F : (sub + 1) * F], o1, biasr[:]
                        )
                nc.sync.dma_start(
                    out_view[:, ch * 4 : (ch + 1) * 4, :], o1c[:]
                )

    nc.compile()
    return nc


_NC_CACHE = {}


def _get_nc(has_bias):
    if has_bias not in _NC_CACHE:
        _NC_CACHE[has_bias] = _build_kernel_module(has_bias)
    return _NC_CACHE[has_bias]


def _step_basis(xh):
    cols = [np.ones_like(xh)]
    for t in np.linspace(-SHIFT_EXT, SHIFT_EXT, R - 1):
        cols.append(np.tanh(BETA * (xh - t)))
    return np.stack(cols, axis=1)


def _fit_K(Ls, Ld, ngrid=160, lam=1e-10):
    """Least-squares fit of K(s,d) = exp(.2(s+d))-exp(s+d) (s+d<0) over
    [-Ls,Ls] x [-Ld,Ld] in the product basis cheb(s) x fourier(d).
    Returns C [R, R] (s-basis x d-basis)."""
    gs = np.linspace(-1.0, 1.0, ngrid)
    S, D = np.meshgrid(gs * Ls, gs * Ld, indexing="ij")
    X = S + D
    K = np.where(X < 0, np.exp(ALPHA * X) - np.exp(X), 0.0)
    Bs = np.polynomial.chebyshev.chebvander(gs, R - 1)
    Bd = _step_basis(gs)

    def pinv(B):
        U, sv, Vt = np.linalg.svd(B, full_matrices=False)
        return (Vt.T * (sv / (sv ** 2 + lam))) @ U.T

    return pinv(Bs) @ K @ pinv(Bd).T


def _make_in_maps(h, w, a_src, a_dst, bias):
    h = np.ascontiguousarray(np.asarray(h, dtype=np.float32))
    w = np.asarray(w, dtype=np.float32)
    a_src = np.asarray(a_src, dtype=np.float32)
    a_dst = np.asarray(a_dst, dtype=np.float32)
    bias = np.asarray(bias, dtype=np.float32)
    has_bias = bool(np.any(bias != 0.0))
    biasr = np.ascontiguousarray(np.broadcast_to(bias[None, :], (128, F)))

    in_maps = []
    for c in range(8):
        head, half = c // 2, c % 2
        ws = w[head] @ a_src[head][:, 0]          # [64]
        wd = w[head] @ a_dst[head][:, 0]
        s_all = h @ ws
        d_all = h @ wd
        Ls = float(np.abs(s_all).max()) * 1.02 + 1e-30
        Ld = float(np.abs(d_all).max()) * 1.02 + 1e-30
        C = _fit_K(Ls, Ld)
        waug = np.concatenate(
            [w[head], (ws / Ls)[:, None], (wd / Ld)[:, None]], axis=1
        ).astype(np.float32)
        # E[b, 32k+a] = scale*C[a, b] (a,b < R); E[R, 32k+R] = scale
        E = np.zeros((MB, 128), dtype=np.float32)
        for k in range(4):
            E[0:R, 32 * k : 32 * k + R] = ND_SCALE * C.T
            E[R, 32 * k + R] = ND_SCALE
        scal = np.broadcast_to(
            np.array([Ls, Ld], dtype=np.float32)[None, :], (128, 2)
        )
        # rotate h so this core's own half sits in j-tiles 0..31
        h_rot = h if half == 0 else np.concatenate([h[HALF:], h[:HALF]], axis=0)
        m = {
            "hfull": np.ascontiguousarray(h_rot),
            "waug": np.ascontiguousarray(waug),
            "emat": E,
            "scal": np.ascontiguousarray(scal),
        }
        if has_bias:
            m["biasr"] = biasr
        in_maps.append(m)
    return has_bias, in_maps


def _run(h, w, a_src, a_dst, bias, trace=False, **trace_kwargs):
    has_bias, in_maps = _make_in_maps(h, w, a_src, a_dst, bias)
    nc = _get_nc(has_bias)
    res = run_bass_kernel_spmd(
        nc, in_maps, core_ids=list(range(8)), trace=trace, **trace_kwargs
    )
    out = np.zeros((BS, NH * F), dtype=np.float32)
    for c in range(8):
        head, half = c // 2, c % 2
        out[half * HALF : (half + 1) * HALF, head * F : (head + 1) * F] = res.results[
            c
        ]["out"]
    return out, res


def kernel(h, w, a_src, a_dst, bias):
    out, _ = _run(h, w, a_src, a_dst, bias, trace=False)
    return out


# revision 27
# speedup vs baseline: 6.4535x; 1.1629x over previous
"""Multi-head graph-attention (GAT) kernel for Trainium2, 8 NeuronCores.

Reference computation (per head):
    h_prime = h @ w[head]                       # [8192, 64]
    s = h_prime @ a_src[head],  d = h_prime @ a_dst[head]
    attn = softmax_j(leaky_relu(s_i + d_j, 0.2))
    out  = attn @ h_prime + bias                # -> [8192, 4*64]

Low-rank reformulation (no O(n^2) work on device):
    W[i,j] = exp(lrelu(s_i + d_j)) = e^{s_i} e^{d_j} + K(s_i, d_j)
  where K(s,d) = exp(0.2(s+d)) - exp(s+d) for s+d < 0, else 0, is a bounded
  continuous function on the (s,d) rectangle covered by the data.  K is
  approximated by a rank-31 product expansion fitted ON THE HOST per head
  (least squares; coefficients and ranges become runtime inputs):
    K(s,d) ~= sum_{a,b} C[a,b] f_a(s/Ls) g_b(d/Ld)
  f_a = Chebyshev T_a (serial DVE recurrence on the small i side),
  g_b = Fourier {1, sin(k w x), cos(k w x)} (30 independent, chain-free
  Sin activations on the scalar engine for the large j side, issued in two
  row-halves so they overlap the h'-building phase).
  With Haug = [h' | 0 0 | 1] and TDaug = [g_0..g_30 | e^d] (j on partitions):
    B      = TDaug^T @ Haug                      # [32, 67]   (PE, O(n R))
    BKaug4 = E^T @ B                             # [128, 67]  one matmul; E is
             host-built: rows 32k+a = sum_b C[a,b] B_gb + e^d row at 32k+31,
             replicated for k=0..3 and pre-scaled by 2^-8
    out^T  = Faug-tile^T @ BKaug4[32k:32k+32]    # [128 i, 67] per i-tile
  where Faug = [T_0..T_30 | e^s], transposed 4 i-tiles at a time (partition
  bands 0/32/64/96 + matching tile_position).  Row 66 of out^T is the softmax
  denominator; the epilogue divides (+bias) and stores.  Max rel err vs the
  reference is ~7e-4 (fp16 operands, fp32 psum accumulation): 25x inside the
  2e-2 gate.

Sharding: 8 cores = 4 heads x 2 row-halves (head parallel + bs row shard).
Each core gets full h (rotated so its own half is first) plus per-head
host-fitted constants; no collectives.
"""

import numpy as np

import concourse.bass as bass
import concourse.tile as tile
from concourse import bacc, mybir
from concourse.bass_utils import run_bass_kernel_spmd
from concourse.masks import make_identity

F32 = mybir.dt.float32
F16 = mybir.dt.float16
AF = mybir.ActivationFunctionType

BS = 8192          # nodes
F = 64             # f_in == f_out
NH = 4             # heads
HALF = BS // 2     # rows per core (row-half)
NT = BS // 128     # 64 j tiles
NTI = HALF // 128  # 32 i tiles
R = 31             # expansion rank per side
MB = R + 1         # TDaug / Faug columns (basis + e^x)
MH = 67            # Haug columns: h'(64), 2 zero, ones
ND_RANK = 20       # number of tanh soft-steps on the d side
BETA = 10.0        # tanh soft-step sharpness (d-side basis)
SHIFT_EXT = 1.1    # shift extent of the tanh steps
ALPHA = 0.2
ND_SCALE = 2.0 ** -8   # folded into E so num/den stay small


def _build_kernel_module(has_bias):
    nc = bacc.Bacc("TRN2", target_bir_lowering=False, debug=False)

    h_d = nc.dram_tensor("hfull", [BS, F], F32, kind="ExternalInput")
    # waug: [w | (w@a_src)/Ls | (w@a_dst)/Ld]
    waug_d = nc.dram_tensor("waug", [F, F + 2], F32, kind="ExternalInput")
    # E: [MB, 128] expansion matrix (fit coeffs -> 4x replicated BKaug rows)
    e_d = nc.dram_tensor("emat", [MB, 128], F32, kind="ExternalInput")
    # scal: [128, 2] (Ls, Ld) replicated across partitions
    scal_d = nc.dram_tensor("scal", [128, 2], F32, kind="ExternalInput")
    if has_bias:
        biasr_d = nc.dram_tensor("biasr", [128, F], F32, kind="ExternalInput")
    out_d = nc.dram_tensor("out", [HALF, F], F32, kind="ExternalOutput")

    with tile.TileContext(nc) as tc:
        with (
            tc.tile_pool(name="const", bufs=1) as cpool,
            tc.tile_pool(name="work", bufs=3) as wpool,
            tc.tile_pool(name="psum", bufs=2, space="PSUM") as ppool,
        ):
            # first h block DMA goes on the queue BEFORE the const DMAs so
            # the PE can start transposing as early as possible
            hview = h_d.ap().rearrange("(a p) f -> p a f", p=128)
            ldb0 = wpool.tile([128, 8 * F], F32, tag="ldb", bufs=4)
            nc.sync.dma_start(ldb0[:], hview[:, 0:8, :])

            # ---------------- constants ----------------
            ident32 = cpool.tile([128, 128], F32)
            make_identity(nc, ident32[:])
            ident16 = cpool.tile([128, 128], F16)
            nc.gpsimd.tensor_copy(ident16[:], ident32[:])

            # waug on partitions 0-63 AND 64-127 (for row-band hp matmuls)
            waug_sb = cpool.tile([128, F + 2], F32)
            nc.sync.dma_start(waug_sb[0:F, :], waug_d.ap())
            nc.sync.dma_start(waug_sb[F:128, :], waug_d.ap())
            waug16 = cpool.tile([128, F + 2], F16)
            nc.gpsimd.tensor_copy(waug16[:], waug_sb[:])

            e_sb = cpool.tile([MB, 128], F32)
            nc.sync.dma_start(e_sb[:], e_d.ap())
            e16 = cpool.tile([MB, 128], F16)

            scal_sb = cpool.tile([128, 2], F32)
            nc.sync.dma_start(scal_sb[:], scal_d.ap())
            if has_bias:
                biasr = cpool.tile([128, F], F32)
                nc.sync.dma_start(biasr[:], biasr_d.ap())

            # ---------------- big SBUF tensors ----------------
            # h^T in 2-tile blocks: block t holds j-tiles (2t, 2t+1) on
            # partition halves 0-63 / 64-127
            hT2 = cpool.tile([128, (NT // 2) * 128], F16)
            Haug = cpool.tile([128, NT * MH], F16)      # [h' | 0 0 | 1]
            Haug3 = Haug[:].rearrange("p (t c) -> p t c", c=MH)
            sd = cpool.tile([128, NT * 2], F32)         # [s^, d^] per j tile
            sd3 = sd[:].rearrange("p (t c) -> p t c", c=2)
            TDaug = cpool.tile([128, NT * MB], F16)     # [fourier(d^) | e^d]
            TDaug3 = TDaug[:].rearrange("p (t c) -> p t c", c=MB)
            Fi = cpool.tile([128, NTI * MB], F16)       # [cheb(s^) | e^s]
            Fi3 = Fi[:].rearrange("p (t c) -> p t c", c=MB)
            S_all = cpool.tile([128, R * NTI], F32)     # cheb fp32 state
            S3 = S_all[:].rearrange("p (b t) -> p b t", t=NTI)
            # Faug^T: block q holds i-tiles 4q..4q+3 on partition bands 32k
            F3t = cpool.tile([128, (NTI // 4) * 128], F16)
            F33 = F3t[:].rearrange("p (t c) -> p t c", c=128)
            BKaug4 = cpool.tile([128, MH], F16)

            def init_memsets():
                nc.gpsimd.memset(Haug3[:, :, F : F + 2], 0.0)
                nc.gpsimd.memset(Haug3[:, :, MH - 1], 1.0)
                nc.gpsimd.memset(TDaug3[:, :, 0], 1.0)
                nc.gpsimd.memset(TDaug3[:, :, 1 + ND_RANK : MB - 1], 0.0)
                nc.gpsimd.memset(Fi3[:, :, 0], 1.0)
                nc.gpsimd.memset(S3[:, 0, :], 1.0)

            def cp(e, out, in_):
                (e.copy if e is nc.scalar else e.tensor_copy)(out, in_)

            # ---------------- phase 1 block worker ----------------
            # PSUM-reading copies only on vector/scalar (GPSIMD has no PSUM
            # access).  eng_ht drains the transposes, eng_hs drains h'.
            def phase1_block(blk, eng_ht, eng_hs, ldb=None):
                if ldb is None:
                    ldb = wpool.tile([128, 8 * F], F32, tag="ldb", bufs=4)
                    nc.sync.dma_start(
                        ldb[:], hview[:, blk * 8 : (blk + 1) * 8, :]
                    )
                ldb16 = wpool.tile([128, 8 * F], F16, tag="ldb16", bufs=4)
                nc.gpsimd.tensor_copy(ldb16[:], ldb[:])
                for g in range(2):
                    jt0 = blk * 8 + g * 4
                    trp2 = ppool.tile([128, 256], F16, tag="mix", bufs=6)
                    for k in range(2):
                        nc.tensor.transpose(
                            trp2[:, k * 128 : (k + 1) * 128],
                            ldb16[:, (g * 4 + 2 * k) * F : (g * 4 + 2 * k + 2) * F],
                            ident16[:],
                        )
                    blk2 = jt0 // 2
                    cp(eng_ht[g], hT2[:, blk2 * 128 : (blk2 + 2) * 128], trp2[:])
                    # per-tile-position psum tiles: mixing tile_positions as
                    # slice-writes of ONE psum tile wedges the PE
                    hp2 = [
                        ppool.tile([128, 2 * (F + 2)], F32, tag="mix", bufs=6,
                                   name=f"hp2_{jt0}_{i}")
                        for i in range(2)
                    ]
                    for k in range(4):
                        jt = jt0 + k
                        lo = (jt % 2) * F
                        dst = hp2[k % 2][:].rearrange(
                            "p (t c) -> p t c", c=F + 2)[:, k // 2, :]
                        nc.tensor.matmul(
                            dst,
                            hT2[lo : lo + F, (jt // 2) * 128 : (jt // 2 + 1) * 128],
                            waug16[lo : lo + F, :],
                            tile_position=(lo, 0),
                        )
                    for i in range(2):
                        h3 = hp2[i][:].rearrange("p (t c) -> p t c", c=F + 2)
                        cp(eng_hs[(g + i) % 2],
                           Haug3[:, jt0 + i : jt0 + 4 : 2, 0:F], h3[:, :, 0:F])
                        cp(eng_hs[(g + i + 1) % 2],
                           sd3[:, jt0 + i : jt0 + 4 : 2, :], h3[:, :, F : F + 2])

            # ---------------- d side: tanh soft-step basis via scalar ACTs --
            # g_b(x) = tanh(BETA * (x - t_b)): one chain-free ACT per column
            shifts = np.linspace(-SHIFT_EXT, SHIFT_EXT, ND_RANK)
            tb_bias = cpool.tile([128, ND_RANK], F32)

            def d_tanh(lo, hi):
                dv = sd3[:, lo:hi, 1]
                nc.scalar.activation(
                    TDaug3[:, lo:hi, MB - 1], dv, AF.Exp, scale=scal_sb[:, 1:2]
                )
                for b in range(ND_RANK):
                    nc.scalar.activation(
                        TDaug3[:, lo:hi, 1 + b], dv, AF.Tanh,
                        scale=BETA, bias=tb_bias[:, b : b + 1],
                    )

            # ---------------- s side: Chebyshev recurrence on DVE ----------
            s_view = sd3[:, 0:NTI, 0]
            x2s = cpool.tile([128, NTI], F32)

            def s_cheb_setup():
                nc.scalar.activation(
                    Fi3[:, :, MB - 1], s_view, AF.Exp, scale=scal_sb[:, 0:1]
                )
                nc.gpsimd.tensor_copy(Fi3[:, :, 1], s_view)
                nc.vector.tensor_copy(S3[:, 1, :], s_view)
                nc.vector.tensor_scalar_mul(x2s[:], s_view, 2.0)

            def s_cheb_chunk(b0, b1):
                for b in range(b0, b1):
                    tmp = wpool.tile([128, NTI], F32, tag="tmps", bufs=2)
                    nc.vector.tensor_mul(tmp[:], x2s[:], S3[:, b - 1, :])
                    nc.vector.tensor_sub(S3[:, b, :], tmp[:], S3[:, b - 2, :])
                    if b % 2 == 0:
                        nc.gpsimd.tensor_copy(
                            Fi3[:, :, b - 1 : b + 1],
                            S3[:, b - 1 : b + 1, :].rearrange("p b t -> p t b"),
                        )

            # ---------------- emit program ----------------
            phase1_block(0, [nc.vector, nc.scalar], [nc.scalar, nc.vector],
                         ldb=ldb0)
            phase1_block(1, [nc.scalar, nc.vector], [nc.vector, nc.scalar])
            init_memsets()
            nc.gpsimd.tensor_copy(e16[:], e_sb[:])
            for b, t in enumerate(shifts):
                nc.gpsimd.memset(tb_bias[:, b : b + 1], float(-BETA * t))
            for blk in range(2, 4):
                et = [nc.vector, nc.scalar] if blk % 2 else [nc.scalar, nc.vector]
                phase1_block(blk, et, et[::-1])
            s_cheb_setup()
            d_tanh(0, 32)               # scalar, overlaps blocks 4-7
            s_chunks = [(2, 9), (9, 16), (16, 23), (23, R)]
            for blk in range(4, 8):
                phase1_block(blk, [nc.vector, nc.vector], [nc.scalar, nc.scalar])
                s_cheb_chunk(*s_chunks[blk - 4])
            d_tanh(32, 64)              # scalar, after block 7's sd lands

            # ---------------- B = TDaug^T @ Haug ----------------
            # split in halves so the Faug transposes can fill the PE while
            # the second-half d basis finishes
            B_ps = ppool.tile([MB, MH], F32, tag="acc", bufs=1)
            for jt in range(32):
                nc.tensor.matmul(
                    B_ps[:], TDaug3[:, jt, :], Haug3[:, jt, :],
                    start=(jt == 0), stop=False,
                )

            # transpose Faug (4 i-tiles per go)
            for q in range(NTI // 4):
                ftp = ppool.tile([128, 128], F16, tag="mix", bufs=6)
                nc.tensor.transpose(
                    ftp[:], Fi3[:, 4 * q : 4 * q + 4, :], ident16[:]
                )
                cp((nc.vector, nc.scalar)[q % 2], F33[:, q, :], ftp[:])

            for jt in range(32, NT):
                nc.tensor.matmul(
                    B_ps[:], TDaug3[:, jt, :], Haug3[:, jt, :],
                    start=False, stop=(jt == NT - 1),
                )
            B16 = cpool.tile([MB, MH], F16)
            nc.vector.tensor_copy(B16[:], B_ps[:])

            # BKaug4 = E^T @ B  (4x replicated, pre-scaled)
            bk_ps = ppool.tile([128, MH], F32, tag="mix", bufs=6)
            nc.tensor.matmul(bk_ps[:], e16[:], B16[:])
            nc.vector.tensor_copy(BKaug4[:], bk_ps[:])

            # ---------------- synthesis + epilogue ----------------
            out_view = out_d.ap().rearrange("(a p) f -> p a f", p=128)
            for ch in range(NTI // 4):
                o1c = wpool.tile([128, 4 * F], F32, tag="o1c", bufs=3)
                for sub in range(4):
                    it = ch * 4 + sub
                    q, k = it // 4, it % 4
                    lo = 32 * k
                    ot_ps = ppool.tile([128, MH], F32, tag="mix", bufs=6)
                    nc.tensor.matmul(
                        ot_ps[:],
                        F33[lo : lo + 32, q, :],
                        BKaug4[lo : lo + 32, :],
                        tile_position=(lo, 0),
                    )
                    rec = wpool.tile([128, 1], F32, tag="rec", bufs=4)
                    nc.vector.reciprocal(rec[:], ot_ps[:, MH - 1 : MH])
                    if has_bias:
                        o1 = wpool.tile([128, F], F32, tag="o1", bufs=4)[:]
                    else:
                        o1 = o1c[:, sub * F : (sub + 1) * F]
                    if sub % 2 == 0:
                        nc.scalar.mul(o1, ot_ps[:, 0:F], rec[:])
                    else:
                        nc.vector.tensor_scalar_mul(o1, ot_ps[:, 0:F], rec[:])
                    if has_bias:
                        nc.gpsimd.tensor_add(
                            o1c[:, sub * F : (sub + 1) * F], o1, biasr[:]
                        )
                nc.sync.dma_start(
                    out_view[:, ch * 4 : (ch + 1) * 4, :], o1c[:]
                )

    nc.compile()
    return nc


_NC_CACHE = {}


def _get_nc(has_bias):
    if has_bias not in _NC_CACHE:
        _NC_CACHE[has_bias] = _build_kernel_module(has_bias)
    return _NC_CACHE[has_bias]


def _step_basis(xh):
    cols = [np.ones_like(xh)]
    for t in np.linspace(-SHIFT_EXT, SHIFT_EXT, ND_RANK):
        cols.append(np.tanh(BETA * (xh - t)))
    return np.stack(cols, axis=1)


def _fit_K(Ls, Ld, ngrid=160, lam=1e-10):
    """Least-squares fit of K(s,d) = exp(.2(s+d))-exp(s+d) (s+d<0) over
    [-Ls,Ls] x [-Ld,Ld] in the product basis cheb(s) x fourier(d).
    Returns C [R, R] (s-basis x d-basis)."""
    gs = np.linspace(-1.0, 1.0, ngrid)
    S, D = np.meshgrid(gs * Ls, gs * Ld, indexing="ij")
    X = S + D
    K = np.where(X < 0, np.exp(ALPHA * X) - np.exp(X), 0.0)
    Bs = np.polynomial.chebyshev.chebvander(gs, R - 1)
    Bd = _step_basis(gs)

    def pinv(B):
        U, sv, Vt = np.linalg.svd(B, full_matrices=False)
        return (Vt.T * (sv / (sv ** 2 + lam))) @ U.T

    return pinv(Bs) @ K @ pinv(Bd).T


def _make_in_maps(h, w, a_src, a_dst, bias):
    h = np.ascontiguousarray(np.asarray(h, dtype=np.float32))
    w = np.asarray(w, dtype=np.float32)
    a_src = np.asarray(a_src, dtype=np.float32)
    a_dst = np.asarray(a_dst, dtype=np.float32)
    bias = np.asarray(bias, dtype=np.float32)
    has_bias = bool(np.any(bias != 0.0))
    biasr = np.ascontiguousarray(np.broadcast_to(bias[None, :], (128, F)))

    in_maps = []
    for c in range(8):
        head, half = c // 2, c % 2
        ws = w[head] @ a_src[head][:, 0]          # [64]
        wd = w[head] @ a_dst[head][:, 0]
        s_all = h @ ws
        d_all = h @ wd
        Ls = float(np.abs(s_all).max()) * 1.02 + 1e-30
        Ld = float(np.abs(d_all).max()) * 1.02 + 1e-30
        C = _fit_K(Ls, Ld)
        waug = np.concatenate(
            [w[head], (ws / Ls)[:, None], (wd / Ld)[:, None]], axis=1
        ).astype(np.float32)
        # E[b, 32k+a] = scale*C[a, b]; unused d rows stay 0; e^d at row R
        E = np.zeros((MB, 128), dtype=np.float32)
        for k in range(4):
            E[0 : 1 + ND_RANK, 32 * k : 32 * k + R] = ND_SCALE * C.T
            E[R, 32 * k + R] = ND_SCALE
        scal = np.broadcast_to(
            np.array([Ls, Ld], dtype=np.float32)[None, :], (128, 2)
        )
        # rotate h so this core's own half sits in j-tiles 0..31
        h_rot = h if half == 0 else np.concatenate([h[HALF:], h[:HALF]], axis=0)
        m = {
            "hfull": np.ascontiguousarray(h_rot),
            "waug": np.ascontiguousarray(waug),
            "emat": E,
            "scal": np.ascontiguousarray(scal),
        }
        if has_bias:
            m["biasr"] = biasr
        in_maps.append(m)
    return has_bias, in_maps


def _run(h, w, a_src, a_dst, bias, trace=False, **trace_kwargs):
    has_bias, in_maps = _make_in_maps(h, w, a_src, a_dst, bias)
    nc = _get_nc(has_bias)
    res = run_bass_kernel_spmd(
        nc, in_maps, core_ids=list(range(8)), trace=trace, **trace_kwargs
    )
    out = np.zeros((BS, NH * F), dtype=np.float32)
    for c in range(8):
        head, half = c // 2, c % 2
        out[half * HALF : (half + 1) * HALF, head * F : (head + 1) * F] = res.results[
            c
        ]["out"]
    return out, res


def kernel(h, w, a_src, a_dst, bias):
    out, _ = _run(h, w, a_src, a_dst, bias, trace=False)
    return out
